# revision 7
# baseline (speedup 1.0000x reference)
"""Trainium2 Bass kernel for nn_HAOQAttention (hierarchical attention with
orthogonal query decomposition), data-parallel over 8 NeuronCores.

Sharding: x is [2, 4096, 1024]; each core takes one contiguous 1024-token
slice (= exactly one GLOBAL_W=1024 window = 8 LOCAL_W=128 windows), so the
whole computation is embarrassingly parallel across cores with replicated
weights. All matmuls run in bf16 on the PE; softmax statistics, layernorm
and accumulations stay in fp32 (PSUM).

The orthogonal projections are folded into the QKV weights on the host
(W_eff = P @ W_in^T, exact algebra), so the device pipeline is:
  xT [d, t] -> qkT [f, t] (features on partitions), v in standard [t, c]
  layout with an interleaved ones-column so the AV matmul emits the softmax
  denominator as PSUM row 64 for free.
  scoresT [j, i] = kT^T @ qT with the two heads of each 128-feature chunk
  issued back-to-back at partition bases 0/64 (PE row-tile concurrency for
  the K=64 contractions); exp via ACT with the additive mask folded in
  (constant +1 bias for fully-strictly-lower column ranges, a {1,e}
  multiplicative triangle for diagonal blocks).
  attnT [din, t] feeds the output projection directly; final LN is done in
  standard [t, d] layout where all reductions are free-dim native.
"""

import sys

sys.path.insert(0, "/opt/trn_rl_repo")

import numpy as np
import ml_dtypes

NBF = ml_dtypes.bfloat16

D = 1024
T = 1024            # tokens per core
_E = float(np.e)

_STATE = {}


def _build_nc():
    from contextlib import ExitStack

    import concourse.mybir as mybir
    from concourse import bacc
    from concourse.tile import TileContext

    bf16 = mybir.dt.bfloat16
    f32 = mybir.dt.float32
    AFT = mybir.ActivationFunctionType

    nc = bacc.Bacc(None, target_bir_lowering=False)

    P = nc.declare_dram_parameter
    xT_d = P("xT", [D, T], bf16, isOutput=False)
    winl_d = P("weffT_l", [D, 3 * D], bf16, isOutput=False)
    wing_d = P("weffT_g", [D, 3 * D], bf16, isOutput=False)
    woutl_d = P("woutT_l", [D, D], bf16, isOutput=False)
    woutg_d = P("woutT_g", [D, D], bf16, isOutput=False)
    qkbl_d = P("qkb_l", [128, 16], f32, isOutput=False)
    qkbg_d = P("qkb_g", [128, 16], f32, isOutput=False)
    vbl_d = P("vb_l", [64, 16], f32, isOutput=False)
    vbg_d = P("vb_g", [64, 16], f32, isOutput=False)
    boutl_d = P("bout_l", [1, D], f32, isOutput=False)
    boutg_d = P("bout_g", [1, D], f32, isOutput=False)
    fw_d = P("fw", [1, 2], f32, isOutput=False)
    gamma_d = P("gamma", [1, D], f32, isOutput=False)
    beta_d = P("beta", [1, D], f32, isOutput=False)
    mask8_d = P("mask8", [128, T], bf16, isOutput=False)
    out_d = P("out", [T, D], f32, isOutput=True)

    with TileContext(nc) as tc, ExitStack() as top:
        const = top.enter_context(tc.tile_pool(name="const", bufs=1))

        mask_sb = const.tile([128, T], bf16)
        nc.sync.dma_start(out=mask_sb, in_=mask8_d[:, :])
        gamma_sb = const.tile([128, D], f32)
        nc.sync.dma_start(out=gamma_sb, in_=gamma_d[0:1, :].to_broadcast([128, D]))
        beta_sb = const.tile([128, D], f32)
        nc.sync.dma_start(out=beta_sb, in_=beta_d[0:1, :].to_broadcast([128, D]))
        qkb = {}
        vb = {}
        for br, (qd, vd) in {"l": (qkbl_d, vbl_d), "g": (qkbg_d, vbg_d)}.items():
            qkb[br] = const.tile([128, 16], f32, tag=f"qkb_{br}", name=f"qkb_{br}")
            nc.sync.dma_start(out=qkb[br], in_=qd[:, :])
            vb[br] = const.tile([64, 16], f32, tag=f"vb_{br}", name=f"vb_{br}")
            nc.sync.dma_start(out=vb[br], in_=vd[:, :])

        # fusion softmax weights (2 entries) computed on partition 0
        fw_sb = const.tile([1, 2], f32)
        nc.sync.dma_start(out=fw_sb, in_=fw_d[:, :])
        fe = const.tile([1, 2], f32)
        nc.scalar.activation(out=fe, in_=fw_sb, func=AFT.Exp)
        fs = const.tile([1, 1], f32)
        nc.vector.tensor_add(fs, fe[:, 0:1], fe[:, 1:2])
        fr = const.tile([1, 1], f32)
        nc.vector.reciprocal(out=fr, in_=fs)
        fwn = const.tile([1, 2], f32)
        nc.vector.tensor_scalar_mul(fwn, fe, scalar1=fr)
        fwb = const.tile([128, 2], f32)
        nc.gpsimd.partition_broadcast(fwb, fwn)
        w_sc = {"l": fwb[:, 0:1], "g": fwb[:, 1:2]}

        # combined output-projection bias: w0*b_out_l + w1*b_out_g, broadcast
        bl_sb = const.tile([1, D], f32)
        nc.sync.dma_start(out=bl_sb, in_=boutl_d[:, :])
        bg_sb = const.tile([1, D], f32)
        nc.sync.dma_start(out=bg_sb, in_=boutg_d[:, :])
        bt0 = const.tile([1, D], f32)
        nc.vector.tensor_scalar_mul(bt0, bl_sb, scalar1=fwn[0:1, 0:1])
        bt1 = const.tile([1, D], f32)
        nc.vector.tensor_scalar_mul(bt1, bg_sb, scalar1=fwn[0:1, 1:2])
        bc0 = const.tile([1, D], f32)
        nc.vector.tensor_add(bc0, bt0, bt1)
        bcomb = const.tile([128, D], f32)
        nc.gpsimd.partition_broadcast(bcomb, bc0)
        eps_sb = const.tile([128, 1], f32)
        nc.vector.memset(eps_sb, 1e-5)

        # input activations, resident for both branches
        xtp = top.enter_context(tc.tile_pool(name="xtp", bufs=1))
        xT_sb = [xtp.tile([128, T], bf16, tag=f"xT{i}", name=f"xT{i}") for i in range(8)]
        for i in range(8):
            nc.sync.dma_start(out=xT_sb[i], in_=xT_d[128 * i:128 * i + 128, :])

        acc_es = ExitStack()
        accp = acc_es.enter_context(tc.tile_pool(name="acc", bufs=1))
        facc = [accp.tile([128, D], f32, tag=f"acc{i}", name=f"acc{i}") for i in range(8)]

        # ---- per-branch: qkv -> windowed attention -> out-proj ----
        for br, win_d, wout_d, is_local in (
            ("l", winl_d, woutl_d, True),
            ("g", wing_d, woutg_d, False),
        ):
            with ExitStack() as brs:
                qkvp = brs.enter_context(tc.tile_pool(name=f"qkv_{br}", bufs=1))
                qkT = [qkvp.tile([128, T], bf16, tag=f"qk{br}{i}", name=f"qk{br}{i}") for i in range(16)]
                vT = [qkvp.tile([128, 16 * 65], bf16, tag=f"v{br}{i}", name=f"v{br}{i}") for i in range(8)]

                with tc.tile_pool(name=f"w_{br}", bufs=1) as wp, \
                     tc.tile_pool(name=f"psB_{br}", bufs=4, space="PSUM") as psB:
                    win_sb = [wp.tile([128, 3 * D], bf16, tag=f"win{br}{i}", name=f"win{br}{i}")
                              for i in range(8)]
                    for i in range(8):
                        nc.sync.dma_start(out=win_sb[i],
                                          in_=win_d[128 * i:128 * i + 128, :])
                    # q,k chunks: qkT[fc] = (W_eff[:, fc])^T @ xT, plus bias
                    for fc in range(16):
                        for th in range(2):
                            ps = psB.tile([128, 512], f32, tag="psB", name="psB")
                            for ec in range(8):
                                nc.tensor.matmul(
                                    ps,
                                    win_sb[ec][:, 128 * fc:128 * fc + 128],
                                    xT_sb[ec][:, 512 * th:512 * th + 512],
                                    start=(ec == 0), stop=(ec == 7))
                            nc.scalar.activation(
                                out=qkT[fc][:, 512 * th:512 * th + 512], in_=ps,
                                func=AFT.Identity, bias=qkb[br][:, fc:fc + 1], scale=1.0)
                    # v in standard [t, c] layout, strided by 65 with ones cols
                    for tci in range(8):
                        v3 = vT[tci].rearrange("p (h c) -> p h c", c=65)
                        nc.vector.memset(v3[:, :, 64:65], 1.0)
                        for ch in range(2):
                            ps = psB.tile([128, 512], f32, tag="psB", name="psB")
                            for ec in range(8):
                                nc.tensor.matmul(
                                    ps,
                                    xT_sb[ec][:, 128 * tci:128 * tci + 128],
                                    win_sb[ec][:, 2 * D + 512 * ch:2 * D + 512 * ch + 512],
                                    start=(ec == 0), stop=(ec == 7))
                            nc.vector.tensor_copy(
                                out=v3[:, 8 * ch:8 * ch + 8, 0:64],
                                in_=ps.rearrange("p (h c) -> p h c", c=64))

                attp = brs.enter_context(tc.tile_pool(name=f"att_{br}", bufs=1))
                attnT = [attp.tile([128, T], bf16, tag=f"at{br}{i}", name=f"at{br}{i}") for i in range(8)]

                def normalize(h, ps_av, tp):
                    """ps_av [65, T]: rows 0:64 unnormalized attn, row 64 den.
                    Drain PSUM to SBUF right away (one fast ACT copy) so the
                    bank frees for the next head pair; the slow reciprocal /
                    broadcast chain then runs off the PE critical path."""
                    av_sb = tp.tile([65, T], f32, tag="av_sb", name="av_sb")
                    nc.scalar.copy(out=av_sb, in_=ps_av)
                    recip_t = tp.tile([65, T], f32, tag="recip", name="recip")
                    nc.vector.reciprocal(out=recip_t[64:65, :],
                                         in_=av_sb[64:65, :])
                    r0 = tp.tile([1, T], f32, tag="r0", name="r0")
                    nc.sync.dma_start(out=r0, in_=recip_t[64:65, :])
                    rb = tp.tile([64, T], f32, tag="rb", name="rb")
                    nc.gpsimd.partition_broadcast(rb, r0)
                    fc = h // 2
                    vb_col = vb[br][:, h:h + 1]
                    if h % 2 == 0:
                        dst = attnT[fc][0:64, :]
                        nc.vector.tensor_mul(dst, av_sb[0:64, :], rb)
                        nc.vector.tensor_scalar_add(dst, dst, scalar1=vb_col)
                    else:
                        stg = tp.tile([64, T], bf16, tag="stg", name="stg")
                        nc.vector.tensor_mul(stg, av_sb[0:64, :], rb)
                        nc.vector.tensor_scalar_add(stg, stg, scalar1=vb_col)
                        nc.sync.dma_start(out=attnT[fc][64:128, :], in_=stg)

                with tc.tile_pool(name=f"tr_{br}", bufs=2) as tp, \
                     tc.tile_pool(name=f"exp_{br}", bufs=1) as ep, \
                     tc.tile_pool(name=f"psS_{br}", bufs=1, space="PSUM") as psS, \
                     tc.tile_pool(name=f"psV_{br}", bufs=1, space="PSUM") as psV:
                    for hp in range(8):
                        fq, fk = hp, 8 + hp
                        kT = [qkT[fk][0:64, :], qkT[fk][64:128, :]]
                        qT = [qkT[fq][0:64, :], qkT[fq][64:128, :]]
                        if is_local:
                            # both parities' score MMs issued adjacently:
                            # K=64 row-tiles at bases 0/64 run concurrently
                            ps_s = [psS.tile([128, T], f32, tag=f"ps_s{p}",
                                             name=f"ps_s{p}") for p in range(2)]
                            for w in range(8):
                                sl = slice(128 * w, 128 * w + 128)
                                for p in range(2):
                                    nc.tensor.matmul(ps_s[p][:, sl], kT[p][:, sl],
                                                     qT[p][:, sl],
                                                     start=True, stop=True)
                            expT = []
                            for p in range(2):
                                eT = ep.tile([128, T], bf16, tag=f"expT{p}",
                                             name=f"expT{p}")
                                nc.scalar.activation(out=eT, in_=ps_s[p],
                                                     func=AFT.Exp, scale=0.125)
                                nc.vector.tensor_mul(eT, eT, mask_sb)
                                expT.append(eT)
                            ps_av = [psV.tile([65, T], f32, tag=f"ps_av{p}",
                                              name=f"ps_av{p}") for p in range(2)]
                            for w in range(8):
                                sl = slice(128 * w, 128 * w + 128)
                                for p in range(2):
                                    h = 2 * hp + p
                                    nc.tensor.matmul(
                                        ps_av[p][:, sl],
                                        vT[w][:, 65 * h:65 * h + 65],
                                        expT[p][:, sl], start=True, stop=True)
                        else:
                            ps_av = [psV.tile([65, T], f32, tag=f"ps_av{p}",
                                              name=f"ps_av{p}") for p in range(2)]
                            for jc in range(8):
                                ps_s = [psS.tile([128, T], f32, tag=f"ps_s{p}",
                                                 name=f"ps_s{p}") for p in range(2)]
                                for ih in range(2):
                                    sl = slice(512 * ih, 512 * ih + 512)
                                    for p in range(2):
                                        nc.tensor.matmul(
                                            ps_s[p][:, sl],
                                            kT[p][:, 128 * jc:128 * jc + 128],
                                            qT[p][:, sl], start=True, stop=True)
                                b = 128 * jc
                                eTj = []
                                for p in range(2):
                                    eT = ep.tile([128, T], bf16, tag=f"eG{p}",
                                                 name=f"eG{p}", bufs=3)
                                    if jc > 0:
                                        nc.scalar.activation(
                                            out=eT[:, 0:b], in_=ps_s[p][:, 0:b],
                                            func=AFT.Exp, scale=0.125, bias=1.0)
                                    nc.scalar.activation(
                                        out=eT[:, b:T], in_=ps_s[p][:, b:T],
                                        func=AFT.Exp, scale=0.125)
                                    nc.vector.tensor_mul(eT[:, b:b + 128],
                                                         eT[:, b:b + 128],
                                                         mask_sb[:, 0:128])
                                    eTj.append(eT)
                                for ih in range(2):
                                    sl = slice(512 * ih, 512 * ih + 512)
                                    for p in range(2):
                                        h = 2 * hp + p
                                        nc.tensor.matmul(
                                            ps_av[p][:, sl],
                                            vT[jc][:, 65 * h:65 * h + 65],
                                            eTj[p][:, sl],
                                            start=(jc == 0), stop=(jc == 7))
                        normalize(2 * hp, ps_av[0], tp)
                        normalize(2 * hp + 1, ps_av[1], tp)

                # out-projection into fused accumulator
                with tc.tile_pool(name=f"wo_{br}", bufs=1) as wo, \
                     tc.tile_pool(name=f"tmp_{br}", bufs=3) as tq, \
                     tc.tile_pool(name=f"psO_{br}", bufs=4, space="PSUM") as psO:
                    wout_sb = [wo.tile([128, D], bf16, tag=f"wo{br}{i}", name=f"wo{br}{i}")
                               for i in range(8)]
                    for i in range(8):
                        nc.sync.dma_start(out=wout_sb[i],
                                          in_=wout_d[128 * i:128 * i + 128, :])
                    for tci in range(8):
                        for oh in range(2):
                            sl = slice(512 * oh, 512 * oh + 512)
                            ps = psO.tile([128, 512], f32, tag="psO", name="psO")
                            for fc in range(8):
                                nc.tensor.matmul(
                                    ps,
                                    attnT[fc][:, 128 * tci:128 * tci + 128],
                                    wout_sb[fc][:, sl],
                                    start=(fc == 0), stop=(fc == 7))
                            if is_local:
                                nc.vector.tensor_scalar_mul(
                                    facc[tci][:, sl], ps, scalar1=w_sc[br])
                            else:
                                tmp = tq.tile([128, 512], f32, tag="tmp", name="tmp")
                                nc.vector.tensor_scalar_mul(tmp, ps, scalar1=w_sc[br])
                                nc.vector.tensor_add(
                                    facc[tci][:, sl], facc[tci][:, sl], tmp)

        # ---- fuse bias + LayerNorm + output ----
        with tc.tile_pool(name="ln", bufs=3) as lp:
            for tci in range(8):
                f = facc[tci]
                nc.vector.tensor_add(f, f, bcomb)
                stats = lp.tile([128, 2, 6], f32, tag="stats", name="stats")
                for sg in range(2):
                    nc.vector.bn_stats(out=stats[:, sg, :],
                                       in_=f[:, 512 * sg:512 * sg + 512])
                mv = lp.tile([128, 2], f32, tag="mv", name="mv")
                nc.vector.bn_aggr(out=mv, in_=stats)
                sq = lp.tile([128, 1], f32, tag="sq", name="sq")
                nc.scalar.activation(out=sq, in_=mv[:, 1:2], func=AFT.Sqrt,
                                     bias=eps_sb, scale=1.0)
                rstd = lp.tile([128, 1], f32, tag="rstd", name="rstd")
                nc.vector.reciprocal(out=rstd, in_=sq)
                o = lp.tile([128, D], f32, tag="o", name="o")
                nc.vector.tensor_scalar(
                    out=o, in0=f, scalar1=mv[:, 0:1], scalar2=rstd,
                    op0=mybir.AluOpType.subtract, op1=mybir.AluOpType.mult)
                nc.vector.tensor_mul(o, o, gamma_sb)
                nc.vector.tensor_add(o, o, beta_sb)
                nc.sync.dma_start(out=out_d[128 * tci:128 * tci + 128, :], in_=o)
        acc_es.close()

    nc.finalize()
    return nc


def _get_nc():
    if "nc" not in _STATE:
        _STATE["nc"] = _build_nc()
    return _STATE["nc"]


def _host_inputs(inputs):
    """Build the per-core in_maps (host-side sharding/layout/constant-folding)."""
    f = lambda a: np.asarray(a, dtype=np.float32)
    x = f(inputs["x"])
    tri = np.where(np.arange(128)[:, None] > np.arange(128)[None, :],
                   np.float32(_E), np.float32(1.0))
    mask8 = np.tile(tri, (1, 8)).astype(NBF)

    def qk_bias(b_in):
        return np.ascontiguousarray(f(b_in)[:2 * D].reshape(16, 128).T)

    def v_bias(b_in):
        return np.ascontiguousarray(f(b_in)[2 * D:].reshape(16, 64).T)

    # fold the orthogonal projection into the qkv weights (exact algebra:
    # qkv = (x @ P) @ W_in^T = x @ (P @ W_in^T))
    weffT_l = (f(inputs["proj_local"]) @ f(inputs["Wl_in"]).T).astype(NBF)
    weffT_g = (f(inputs["proj_global"]) @ f(inputs["Wg_in"]).T).astype(NBF)

    common = {
        "weffT_l": weffT_l,
        "weffT_g": weffT_g,
        "woutT_l": np.ascontiguousarray(f(inputs["Wl_out"]).T).astype(NBF),
        "woutT_g": np.ascontiguousarray(f(inputs["Wg_out"]).T).astype(NBF),
        "qkb_l": qk_bias(inputs["bl_in"]),
        "qkb_g": qk_bias(inputs["bg_in"]),
        "vb_l": v_bias(inputs["bl_in"]),
        "vb_g": v_bias(inputs["bg_in"]),
        "bout_l": f(inputs["bl_out"]).reshape(1, D),
        "bout_g": f(inputs["bg_out"]).reshape(1, D),
        "fw": f(inputs["fusion_w"]).reshape(1, 2),
        "gamma": f(inputs["ln_gamma"]).reshape(1, D),
        "beta": f(inputs["ln_beta"]).reshape(1, D),
        "mask8": mask8,
    }
    in_maps = []
    for core in range(8):
        b, t0 = core // 4, (core % 4) * T
        xT = np.ascontiguousarray(x[b, t0:t0 + T, :].T).astype(NBF)
        in_maps.append({**common, "xT": xT})
    return in_maps


def _run(inputs, trace=False):
    from concourse.bass_utils import run_bass_kernel_spmd

    nc = _get_nc()
    in_maps = _host_inputs(inputs)
    res = run_bass_kernel_spmd(nc, in_maps, core_ids=list(range(8)), trace=trace)
    x = np.asarray(inputs["x"])
    out = np.empty((2, 4096, D), np.float32)
    for core in range(8):
        b, t0 = core // 4, (core % 4) * T
        out[b, t0:t0 + T, :] = res.results[core]["out"]
    return out, res


def kernel(**inputs) -> np.ndarray:
    out, _ = _run(inputs)
    return out


# revision 8
# speedup vs baseline: 1.1560x; 1.1560x over previous
"""Trainium2 Bass kernel for nn_HAOQAttention (hierarchical attention with
orthogonal query decomposition), data-parallel over 8 NeuronCores.

Sharding: x is [2, 4096, 1024]; each core takes one contiguous 1024-token
slice (= exactly one GLOBAL_W=1024 window = 8 LOCAL_W=128 windows), so the
whole computation is embarrassingly parallel across cores with replicated
weights. All matmuls run in bf16 on the PE; softmax statistics, layernorm
and accumulations stay in fp32 (PSUM).

The orthogonal projections are folded into the QKV weights on the host
(W_eff = P @ W_in^T, exact algebra), so the device pipeline is:
  xT [d, t] -> qkT [f, t] (features on partitions), v in standard [t, c]
  layout with an interleaved ones-column so the AV matmul emits the softmax
  denominator as PSUM row 64 for free.
  scoresT [j, i] = kT^T @ qT with the two heads of each 128-feature chunk
  issued back-to-back at partition bases 0/64 (PE row-tile concurrency for
  the K=64 contractions); exp via ACT with the additive mask folded in
  (constant +1 bias for fully-strictly-lower column ranges, a {1,e}
  multiplicative triangle for diagonal blocks).
  attnT [din, t] feeds the output projection directly; final LN is done in
  standard [t, d] layout where all reductions are free-dim native.
"""

import sys

sys.path.insert(0, "/opt/trn_rl_repo")

import numpy as np
import ml_dtypes

NBF = ml_dtypes.bfloat16

D = 1024
T = 1024            # tokens per core
_E = float(np.e)

_STATE = {}


def _build_nc():
    from contextlib import ExitStack

    import concourse.mybir as mybir
    from concourse import bacc
    from concourse.tile import TileContext

    bf16 = mybir.dt.bfloat16
    f32 = mybir.dt.float32
    AFT = mybir.ActivationFunctionType

    nc = bacc.Bacc(None, target_bir_lowering=False)

    P = nc.declare_dram_parameter
    xT_d = P("xT", [D, T], bf16, isOutput=False)
    winl_d = P("weffT_l", [D, 3 * D], bf16, isOutput=False)
    wing_d = P("weffT_g", [D, 3 * D], bf16, isOutput=False)
    woutl_d = P("woutT_l", [D, D], bf16, isOutput=False)
    woutg_d = P("woutT_g", [D, D], bf16, isOutput=False)
    qkbl_d = P("qkb_l", [128, 16], f32, isOutput=False)
    qkbg_d = P("qkb_g", [128, 16], f32, isOutput=False)
    vbl_d = P("vb_l", [64, 16], f32, isOutput=False)
    vbg_d = P("vb_g", [64, 16], f32, isOutput=False)
    boutl_d = P("bout_l", [1, D], f32, isOutput=False)
    boutg_d = P("bout_g", [1, D], f32, isOutput=False)
    fw_d = P("fw", [1, 2], f32, isOutput=False)
    gamma_d = P("gamma", [1, D], f32, isOutput=False)
    beta_d = P("beta", [1, D], f32, isOutput=False)
    mask8_d = P("mask8", [128, T], bf16, isOutput=False)
    out_d = P("out", [T, D], f32, isOutput=True)

    with TileContext(nc) as tc, ExitStack() as top:
        const = top.enter_context(tc.tile_pool(name="const", bufs=1))

        mask_sb = const.tile([128, T], bf16)
        nc.sync.dma_start(out=mask_sb, in_=mask8_d[:, :])
        gamma_sb = const.tile([128, D], f32)
        nc.sync.dma_start(out=gamma_sb, in_=gamma_d[0:1, :].to_broadcast([128, D]))
        beta_sb = const.tile([128, D], f32)
        nc.sync.dma_start(out=beta_sb, in_=beta_d[0:1, :].to_broadcast([128, D]))
        qkb = {}
        vb = {}
        for br, (qd, vd) in {"l": (qkbl_d, vbl_d), "g": (qkbg_d, vbg_d)}.items():
            qkb[br] = const.tile([128, 16], f32, tag=f"qkb_{br}", name=f"qkb_{br}")
            nc.sync.dma_start(out=qkb[br], in_=qd[:, :])
            vb[br] = const.tile([64, 16], f32, tag=f"vb_{br}", name=f"vb_{br}")
            nc.sync.dma_start(out=vb[br], in_=vd[:, :])

        # fusion softmax weights (2 entries) computed on partition 0
        fw_sb = const.tile([1, 2], f32)
        nc.sync.dma_start(out=fw_sb, in_=fw_d[:, :])
        fe = const.tile([1, 2], f32)
        nc.scalar.activation(out=fe, in_=fw_sb, func=AFT.Exp)
        fs = const.tile([1, 1], f32)
        nc.vector.tensor_add(fs, fe[:, 0:1], fe[:, 1:2])
        fr = const.tile([1, 1], f32)
        nc.vector.reciprocal(out=fr, in_=fs)
        fwn = const.tile([1, 2], f32)
        nc.vector.tensor_scalar_mul(fwn, fe, scalar1=fr)
        fwb = const.tile([128, 2], f32)
        nc.gpsimd.partition_broadcast(fwb, fwn)
        w_sc = {"l": fwb[:, 0:1], "g": fwb[:, 1:2]}

        # combined output-projection bias: w0*b_out_l + w1*b_out_g, broadcast
        bl_sb = const.tile([1, D], f32)
        nc.sync.dma_start(out=bl_sb, in_=boutl_d[:, :])
        bg_sb = const.tile([1, D], f32)
        nc.sync.dma_start(out=bg_sb, in_=boutg_d[:, :])
        bt0 = const.tile([1, D], f32)
        nc.vector.tensor_scalar_mul(bt0, bl_sb, scalar1=fwn[0:1, 0:1])
        bt1 = const.tile([1, D], f32)
        nc.vector.tensor_scalar_mul(bt1, bg_sb, scalar1=fwn[0:1, 1:2])
        bc0 = const.tile([1, D], f32)
        nc.vector.tensor_add(bc0, bt0, bt1)
        bcomb = const.tile([128, D], f32)
        nc.gpsimd.partition_broadcast(bcomb, bc0)
        eps_sb = const.tile([128, 1], f32)
        nc.vector.memset(eps_sb, 1e-5)

        # DRAM scratch for the per-head denominator reshape bounce
        dramp = top.enter_context(tc.tile_pool(name="dram", bufs=2, space="DRAM"))

        # input activations, resident for both branches
        xtp = top.enter_context(tc.tile_pool(name="xtp", bufs=1))
        xT_sb = [xtp.tile([128, T], bf16, tag=f"xT{i}", name=f"xT{i}") for i in range(8)]
        for i in range(8):
            nc.sync.dma_start(out=xT_sb[i], in_=xT_d[128 * i:128 * i + 128, :])

        acc_es = ExitStack()
        accp = acc_es.enter_context(tc.tile_pool(name="acc", bufs=1))
        facc = [accp.tile([128, D], f32, tag=f"acc{i}", name=f"acc{i}") for i in range(8)]

        # ---- per-branch: qkv -> windowed attention -> out-proj ----
        for br, win_d, wout_d, is_local in (
            ("l", winl_d, woutl_d, True),
            ("g", wing_d, woutg_d, False),
        ):
            with ExitStack() as brs:
                qkvp = brs.enter_context(tc.tile_pool(name=f"qkv_{br}", bufs=1))
                qkT = [qkvp.tile([128, T], bf16, tag=f"qk{br}{i}", name=f"qk{br}{i}") for i in range(16)]
                vT = [qkvp.tile([128, 16 * 65], bf16, tag=f"v{br}{i}", name=f"v{br}{i}") for i in range(8)]

                with tc.tile_pool(name=f"w_{br}", bufs=1) as wp, \
                     tc.tile_pool(name=f"psB_{br}", bufs=4, space="PSUM") as psB:
                    win_sb = [wp.tile([128, 3 * D], bf16, tag=f"win{br}{i}", name=f"win{br}{i}")
                              for i in range(8)]
                    for i in range(8):
                        nc.sync.dma_start(out=win_sb[i],
                                          in_=win_d[128 * i:128 * i + 128, :])
                    # q,k chunks: qkT[fc] = (W_eff[:, fc])^T @ xT, plus bias
                    for fc in range(16):
                        for th in range(2):
                            ps = psB.tile([128, 512], f32, tag="psB", name="psB")
                            for ec in range(8):
                                nc.tensor.matmul(
                                    ps,
                                    win_sb[ec][:, 128 * fc:128 * fc + 128],
                                    xT_sb[ec][:, 512 * th:512 * th + 512],
                                    start=(ec == 0), stop=(ec == 7))
                            nc.scalar.activation(
                                out=qkT[fc][:, 512 * th:512 * th + 512], in_=ps,
                                func=AFT.Identity, bias=qkb[br][:, fc:fc + 1], scale=1.0)
                    # v in standard [t, c] layout, strided by 65 with ones cols
                    for tci in range(8):
                        v3 = vT[tci].rearrange("p (h c) -> p h c", c=65)
                        nc.vector.memset(v3[:, :, 64:65], 1.0)
                        for ch in range(2):
                            ps = psB.tile([128, 512], f32, tag="psB", name="psB")
                            for ec in range(8):
                                nc.tensor.matmul(
                                    ps,
                                    xT_sb[ec][:, 128 * tci:128 * tci + 128],
                                    win_sb[ec][:, 2 * D + 512 * ch:2 * D + 512 * ch + 512],
                                    start=(ec == 0), stop=(ec == 7))
                            nc.vector.tensor_copy(
                                out=v3[:, 8 * ch:8 * ch + 8, 0:64],
                                in_=ps.rearrange("p (h c) -> p h c", c=64))

                attp = brs.enter_context(tc.tile_pool(name=f"att_{br}", bufs=1))
                attnT = [attp.tile([128, T], bf16, tag=f"at{br}{i}", name=f"at{br}{i}") for i in range(8)]

                def normalize(h, ps_av, tp):
                    """ps_av [65, T]: rows 0:64 unnormalized attn, row 64 den.
                    Drain PSUM to SBUF right away (one fast ACT copy) so the
                    bank frees for the next head pair. The reciprocal runs on
                    a DRAM-bounced [128, 8] reshape of the den row: 8 elems
                    per DVE lane (~70ns) instead of 1024 on one lane (5.1us,
                    which would block the strict-FIFO DVE queue and stall the
                    whole pipeline). The recip row then DMA-broadcasts from
                    DRAM into the [64, T] multiplier tile directly."""
                    av_sb = tp.tile([65, T], f32, tag="av_sb", name="av_sb")
                    nc.scalar.copy(out=av_sb, in_=ps_av)
                    dA = dramp.tile([1, T], f32, tag="dA", name="dA")
                    nc.sync.dma_start(out=dA, in_=av_sb[64:65, :])
                    d2 = tp.tile([128, 8], f32, tag="d2", name="d2")
                    nc.sync.dma_start(
                        out=d2, in_=dA.rearrange("o (p c) -> (o p) c", p=128))
                    d2r = tp.tile([128, 8], f32, tag="d2r", name="d2r")
                    nc.vector.reciprocal(out=d2r, in_=d2)
                    dB = dramp.tile([1, T], f32, tag="dB", name="dB")
                    nc.sync.dma_start(
                        out=dB.rearrange("o (p c) -> (o p) c", p=128), in_=d2r)
                    rb = tp.tile([64, T], f32, tag="rb", name="rb")
                    nc.sync.dma_start(out=rb, in_=dB[0:1, :].to_broadcast([64, T]))
                    fc = h // 2
                    vb_col = vb[br][:, h:h + 1]
                    if h % 2 == 0:
                        dst = attnT[fc][0:64, :]
                        nc.vector.tensor_mul(dst, av_sb[0:64, :], rb)
                        nc.vector.tensor_scalar_add(dst, dst, scalar1=vb_col)
                    else:
                        stg = tp.tile([64, T], bf16, tag="stg", name="stg")
                        nc.vector.tensor_mul(stg, av_sb[0:64, :], rb)
                        nc.vector.tensor_scalar_add(stg, stg, scalar1=vb_col)
                        nc.sync.dma_start(out=attnT[fc][64:128, :], in_=stg)

                with tc.tile_pool(name=f"tr_{br}", bufs=2) as tp, \
                     tc.tile_pool(name=f"exp_{br}", bufs=1) as ep, \
                     tc.tile_pool(name=f"psS_{br}", bufs=1, space="PSUM") as psS, \
                     tc.tile_pool(name=f"psV_{br}", bufs=1, space="PSUM") as psV:
                    for hp in range(8):
                        fq, fk = hp, 8 + hp
                        kT = [qkT[fk][0:64, :], qkT[fk][64:128, :]]
                        qT = [qkT[fq][0:64, :], qkT[fq][64:128, :]]
                        if is_local:
                            # both parities' score MMs issued adjacently:
                            # K=64 row-tiles at bases 0/64 run concurrently
                            ps_s = [psS.tile([128, T], f32, tag=f"ps_s{p}",
                                             name=f"ps_s{p}") for p in range(2)]
                            for w in range(8):
                                sl = slice(128 * w, 128 * w + 128)
                                for p in range(2):
                                    nc.tensor.matmul(ps_s[p][:, sl], kT[p][:, sl],
                                                     qT[p][:, sl],
                                                     start=True, stop=True)
                            expT = []
                            for p in range(2):
                                eT = ep.tile([128, T], bf16, tag=f"expT{p}",
                                             name=f"expT{p}")
                                nc.scalar.activation(out=eT, in_=ps_s[p],
                                                     func=AFT.Exp, scale=0.125)
                                nc.vector.tensor_mul(eT, eT, mask_sb)
                                expT.append(eT)
                            ps_av = [psV.tile([65, T], f32, tag=f"ps_av{p}",
                                              name=f"ps_av{p}") for p in range(2)]
                            for w in range(8):
                                sl = slice(128 * w, 128 * w + 128)
                                for p in range(2):
                                    h = 2 * hp + p
                                    nc.tensor.matmul(
                                        ps_av[p][:, sl],
                                        vT[w][:, 65 * h:65 * h + 65],
                                        expT[p][:, sl], start=True, stop=True)
                        else:
                            ps_av = [psV.tile([65, T], f32, tag=f"ps_av{p}",
                                              name=f"ps_av{p}") for p in range(2)]
                            for jc in range(8):
                                ps_s = [psS.tile([128, T], f32, tag=f"ps_s{p}",
                                                 name=f"ps_s{p}") for p in range(2)]
                                for ih in range(2):
                                    sl = slice(512 * ih, 512 * ih + 512)
                                    for p in range(2):
                                        nc.tensor.matmul(
                                            ps_s[p][:, sl],
                                            kT[p][:, 128 * jc:128 * jc + 128],
                                            qT[p][:, sl], start=True, stop=True)
                                b = 128 * jc
                                eTj = []
                                for p in range(2):
                                    eT = ep.tile([128, T], bf16, tag=f"eG{p}",
                                                 name=f"eG{p}", bufs=3)
                                    if jc > 0:
                                        nc.scalar.activation(
                                            out=eT[:, 0:b], in_=ps_s[p][:, 0:b],
                                            func=AFT.Exp, scale=0.125, bias=1.0)
                                    nc.scalar.activation(
                                        out=eT[:, b:T], in_=ps_s[p][:, b:T],
                                        func=AFT.Exp, scale=0.125)
                                    nc.vector.tensor_mul(eT[:, b:b + 128],
                                                         eT[:, b:b + 128],
                                                         mask_sb[:, 0:128])
                                    eTj.append(eT)
                                for ih in range(2):
                                    sl = slice(512 * ih, 512 * ih + 512)
                                    for p in range(2):
                                        h = 2 * hp + p
                                        nc.tensor.matmul(
                                            ps_av[p][:, sl],
                                            vT[jc][:, 65 * h:65 * h + 65],
                                            eTj[p][:, sl],
                                            start=(jc == 0), stop=(jc == 7))
                        normalize(2 * hp, ps_av[0], tp)
                        normalize(2 * hp + 1, ps_av[1], tp)

                # out-projection into fused accumulator
                with tc.tile_pool(name=f"wo_{br}", bufs=1) as wo, \
                     tc.tile_pool(name=f"tmp_{br}", bufs=3) as tq, \
                     tc.tile_pool(name=f"psO_{br}", bufs=4, space="PSUM") as psO:
                    wout_sb = [wo.tile([128, D], bf16, tag=f"wo{br}{i}", name=f"wo{br}{i}")
                               for i in range(8)]
                    for i in range(8):
                        nc.sync.dma_start(out=wout_sb[i],
                                          in_=wout_d[128 * i:128 * i + 128, :])
                    for tci in range(8):
                        for oh in range(2):
                            sl = slice(512 * oh, 512 * oh + 512)
                            ps = psO.tile([128, 512], f32, tag="psO", name="psO")
                            for fc in range(8):
                                nc.tensor.matmul(
                                    ps,
                                    attnT[fc][:, 128 * tci:128 * tci + 128],
                                    wout_sb[fc][:, sl],
                                    start=(fc == 0), stop=(fc == 7))
                            if is_local:
                                nc.vector.tensor_scalar_mul(
                                    facc[tci][:, sl], ps, scalar1=w_sc[br])
                            else:
                                tmp = tq.tile([128, 512], f32, tag="tmp", name="tmp")
                                nc.vector.tensor_scalar_mul(tmp, ps, scalar1=w_sc[br])
                                nc.vector.tensor_add(
                                    facc[tci][:, sl], facc[tci][:, sl], tmp)

        # ---- fuse bias + LayerNorm + output ----
        with tc.tile_pool(name="ln", bufs=3) as lp:
            for tci in range(8):
                f = facc[tci]
                nc.vector.tensor_add(f, f, bcomb)
                stats = lp.tile([128, 2, 6], f32, tag="stats", name="stats")
                for sg in range(2):
                    nc.vector.bn_stats(out=stats[:, sg, :],
                                       in_=f[:, 512 * sg:512 * sg + 512])
                mv = lp.tile([128, 2], f32, tag="mv", name="mv")
                nc.vector.bn_aggr(out=mv, in_=stats)
                sq = lp.tile([128, 1], f32, tag="sq", name="sq")
                nc.scalar.activation(out=sq, in_=mv[:, 1:2], func=AFT.Sqrt,
                                     bias=eps_sb, scale=1.0)
                rstd = lp.tile([128, 1], f32, tag="rstd", name="rstd")
                nc.vector.reciprocal(out=rstd, in_=sq)
                o = lp.tile([128, D], f32, tag="o", name="o")
                nc.vector.tensor_scalar(
                    out=o, in0=f, scalar1=mv[:, 0:1], scalar2=rstd,
                    op0=mybir.AluOpType.subtract, op1=mybir.AluOpType.mult)
                nc.vector.tensor_mul(o, o, gamma_sb)
                nc.vector.tensor_add(o, o, beta_sb)
                nc.sync.dma_start(out=out_d[128 * tci:128 * tci + 128, :], in_=o)
        acc_es.close()

    nc.finalize()
    return nc


def _get_nc():
    if "nc" not in _STATE:
        _STATE["nc"] = _build_nc()
    return _STATE["nc"]


def _host_inputs(inputs):
    """Build the per-core in_maps (host-side sharding/layout/constant-folding)."""
    f = lambda a: np.asarray(a, dtype=np.float32)
    x = f(inputs["x"])
    tri = np.where(np.arange(128)[:, None] > np.arange(128)[None, :],
                   np.float32(_E), np.float32(1.0))
    mask8 = np.tile(tri, (1, 8)).astype(NBF)

    def qk_bias(b_in):
        return np.ascontiguousarray(f(b_in)[:2 * D].reshape(16, 128).T)

    def v_bias(b_in):
        return np.ascontiguousarray(f(b_in)[2 * D:].reshape(16, 64).T)

    # fold the orthogonal projection into the qkv weights (exact algebra:
    # qkv = (x @ P) @ W_in^T = x @ (P @ W_in^T))
    weffT_l = (f(inputs["proj_local"]) @ f(inputs["Wl_in"]).T).astype(NBF)
    weffT_g = (f(inputs["proj_global"]) @ f(inputs["Wg_in"]).T).astype(NBF)

    common = {
        "weffT_l": weffT_l,
        "weffT_g": weffT_g,
        "woutT_l": np.ascontiguousarray(f(inputs["Wl_out"]).T).astype(NBF),
        "woutT_g": np.ascontiguousarray(f(inputs["Wg_out"]).T).astype(NBF),
        "qkb_l": qk_bias(inputs["bl_in"]),
        "qkb_g": qk_bias(inputs["bg_in"]),
        "vb_l": v_bias(inputs["bl_in"]),
        "vb_g": v_bias(inputs["bg_in"]),
        "bout_l": f(inputs["bl_out"]).reshape(1, D),
        "bout_g": f(inputs["bg_out"]).reshape(1, D),
        "fw": f(inputs["fusion_w"]).reshape(1, 2),
        "gamma": f(inputs["ln_gamma"]).reshape(1, D),
        "beta": f(inputs["ln_beta"]).reshape(1, D),
        "mask8": mask8,
    }
    in_maps = []
    for core in range(8):
        b, t0 = core // 4, (core % 4) * T
        xT = np.ascontiguousarray(x[b, t0:t0 + T, :].T).astype(NBF)
        in_maps.append({**common, "xT": xT})
    return in_maps


def _run(inputs, trace=False):
    from concourse.bass_utils import run_bass_kernel_spmd

    nc = _get_nc()
    in_maps = _host_inputs(inputs)
    res = run_bass_kernel_spmd(nc, in_maps, core_ids=list(range(8)), trace=trace)
    x = np.asarray(inputs["x"])
    out = np.empty((2, 4096, D), np.float32)
    for core in range(8):
        b, t0 = core // 4, (core % 4) * T
        out[b, t0:t0 + T, :] = res.results[core]["out"]
    return out, res


def kernel(**inputs) -> np.ndarray:
    out, _ = _run(inputs)
    return out


# revision 9
# speedup vs baseline: 1.2231x; 1.0581x over previous
"""Trainium2 Bass kernel for nn_HAOQAttention (hierarchical attention with
orthogonal query decomposition), data-parallel over 8 NeuronCores.

Sharding: x is [2, 4096, 1024]; each core takes one contiguous 1024-token
slice (= exactly one GLOBAL_W=1024 window = 8 LOCAL_W=128 windows), so the
whole computation is embarrassingly parallel across cores with replicated
weights. All matmuls run in bf16 on the PE; softmax statistics, layernorm
and accumulations stay in fp32 (PSUM).

The orthogonal projections are folded into the QKV weights on the host
(W_eff = P @ W_in^T, exact algebra), so the device pipeline is:
  xT [d, t] -> qkT [f, t] (features on partitions), v in standard [t, c]
  layout with an interleaved ones-column so the AV matmul emits the softmax
  denominator as PSUM row 64 for free.
  scoresT [j, i] = kT^T @ qT with the two heads of each 128-feature chunk
  issued back-to-back at partition bases 0/64 (PE row-tile concurrency for
  the K=64 contractions); exp via ACT with the additive mask folded in
  (constant +1 bias for fully-strictly-lower column ranges, a {1,e}
  multiplicative triangle for diagonal blocks).
  attnT [din, t] feeds the output projection directly; final LN is done in
  standard [t, d] layout where all reductions are free-dim native.
"""

import sys

sys.path.insert(0, "/opt/trn_rl_repo")

import numpy as np
import ml_dtypes

NBF = ml_dtypes.bfloat16

D = 1024
T = 1024            # tokens per core
_E = float(np.e)

_STATE = {}


def _build_nc():
    from contextlib import ExitStack

    import concourse.mybir as mybir
    from concourse import bacc
    from concourse.tile import TileContext

    bf16 = mybir.dt.bfloat16
    f32 = mybir.dt.float32
    AFT = mybir.ActivationFunctionType

    nc = bacc.Bacc(None, target_bir_lowering=False)

    P = nc.declare_dram_parameter
    xT_d = P("xT", [D, T], bf16, isOutput=False)
    winl_d = P("weffT_l", [D, 3 * D], bf16, isOutput=False)
    wing_d = P("weffT_g", [D, 3 * D], bf16, isOutput=False)
    woutl_d = P("woutT_l", [D, D], bf16, isOutput=False)
    woutg_d = P("woutT_g", [D, D], bf16, isOutput=False)
    qkbl_d = P("qkb_l", [128, 16], f32, isOutput=False)
    qkbg_d = P("qkb_g", [128, 16], f32, isOutput=False)
    vbl_d = P("vb_l", [64, 16], f32, isOutput=False)
    vbg_d = P("vb_g", [64, 16], f32, isOutput=False)
    boutl_d = P("bout_l", [1, D], f32, isOutput=False)
    boutg_d = P("bout_g", [1, D], f32, isOutput=False)
    fw_d = P("fw", [1, 2], f32, isOutput=False)
    gamma_d = P("gamma", [1, D], f32, isOutput=False)
    beta_d = P("beta", [1, D], f32, isOutput=False)
    mask8_d = P("mask8", [128, T], bf16, isOutput=False)
    out_d = P("out", [T, D], f32, isOutput=True)

    with TileContext(nc) as tc, ExitStack() as top:
        const = top.enter_context(tc.tile_pool(name="const", bufs=1))

        mask_sb = const.tile([128, T], bf16)
        nc.sync.dma_start(out=mask_sb, in_=mask8_d[:, :])
        gamma_sb = const.tile([128, D], f32)
        nc.sync.dma_start(out=gamma_sb, in_=gamma_d[0:1, :].to_broadcast([128, D]))
        beta_sb = const.tile([128, D], f32)
        nc.sync.dma_start(out=beta_sb, in_=beta_d[0:1, :].to_broadcast([128, D]))
        qkb = {}
        vb = {}
        for br, (qd, vd) in {"l": (qkbl_d, vbl_d), "g": (qkbg_d, vbg_d)}.items():
            qkb[br] = const.tile([128, 16], f32, tag=f"qkb_{br}", name=f"qkb_{br}")
            nc.sync.dma_start(out=qkb[br], in_=qd[:, :])
            vb[br] = const.tile([64, 16], f32, tag=f"vb_{br}", name=f"vb_{br}")
            nc.sync.dma_start(out=vb[br], in_=vd[:, :])

        # fusion softmax weights (2 entries) computed on partition 0
        fw_sb = const.tile([1, 2], f32)
        nc.sync.dma_start(out=fw_sb, in_=fw_d[:, :])
        fe = const.tile([1, 2], f32)
        nc.scalar.activation(out=fe, in_=fw_sb, func=AFT.Exp)
        fs = const.tile([1, 1], f32)
        nc.vector.tensor_add(fs, fe[:, 0:1], fe[:, 1:2])
        fr = const.tile([1, 1], f32)
        nc.vector.reciprocal(out=fr, in_=fs)
        fwn = const.tile([1, 2], f32)
        nc.vector.tensor_scalar_mul(fwn, fe, scalar1=fr)
        fwb = const.tile([128, 2], f32)
        nc.gpsimd.partition_broadcast(fwb, fwn)
        w_sc = {"l": fwb[:, 0:1], "g": fwb[:, 1:2]}

        # combined output-projection bias: w0*b_out_l + w1*b_out_g, broadcast
        bl_sb = const.tile([1, D], f32)
        nc.sync.dma_start(out=bl_sb, in_=boutl_d[:, :])
        bg_sb = const.tile([1, D], f32)
        nc.sync.dma_start(out=bg_sb, in_=boutg_d[:, :])
        bt0 = const.tile([1, D], f32)
        nc.vector.tensor_scalar_mul(bt0, bl_sb, scalar1=fwn[0:1, 0:1])
        bt1 = const.tile([1, D], f32)
        nc.vector.tensor_scalar_mul(bt1, bg_sb, scalar1=fwn[0:1, 1:2])
        bc0 = const.tile([1, D], f32)
        nc.vector.tensor_add(bc0, bt0, bt1)
        bcomb = const.tile([128, D], f32)
        nc.gpsimd.partition_broadcast(bcomb, bc0)
        eps_sb = const.tile([128, 1], f32)
        nc.vector.memset(eps_sb, 1e-5)

        # DRAM scratch for the per-head denominator reshape bounce
        dramp = top.enter_context(tc.tile_pool(name="dram", bufs=2, space="DRAM"))

        # input activations, resident for both branches
        xtp = top.enter_context(tc.tile_pool(name="xtp", bufs=1))
        xT_sb = [xtp.tile([128, T], bf16, tag=f"xT{i}", name=f"xT{i}") for i in range(8)]
        for i in range(8):
            nc.sync.dma_start(out=xT_sb[i], in_=xT_d[128 * i:128 * i + 128, :])

        acc_es = ExitStack()
        accp = acc_es.enter_context(tc.tile_pool(name="acc", bufs=1))
        facc = [accp.tile([128, D], f32, tag=f"acc{i}", name=f"acc{i}") for i in range(8)]

        # ---- per-branch: qkv -> windowed attention -> out-proj ----
        for br, win_d, wout_d, is_local in (
            ("l", winl_d, woutl_d, True),
            ("g", wing_d, woutg_d, False),
        ):
            with ExitStack() as brs:
                qkvp = brs.enter_context(tc.tile_pool(name=f"qkv_{br}", bufs=1))
                qkT = [qkvp.tile([128, T], bf16, tag=f"qk{br}{i}", name=f"qk{br}{i}") for i in range(16)]
                vT = [qkvp.tile([128, 16 * 65], bf16, tag=f"v{br}{i}", name=f"v{br}{i}") for i in range(8)]

                with tc.tile_pool(name=f"w_{br}", bufs=1) as wp, \
                     tc.tile_pool(name=f"psB_{br}", bufs=4, space="PSUM") as psB:
                    win_sb = [wp.tile([128, 3 * D], bf16, tag=f"win{br}{i}", name=f"win{br}{i}")
                              for i in range(8)]
                    for i in range(8):
                        nc.sync.dma_start(out=win_sb[i],
                                          in_=win_d[128 * i:128 * i + 128, :])
                    # q,k chunks: qkT[fc] = (W_eff[:, fc])^T @ xT, plus bias
                    for fc in range(16):
                        for th in range(2):
                            ps = psB.tile([128, 512], f32, tag="psB", name="psB")
                            for ec in range(8):
                                nc.tensor.matmul(
                                    ps,
                                    win_sb[ec][:, 128 * fc:128 * fc + 128],
                                    xT_sb[ec][:, 512 * th:512 * th + 512],
                                    start=(ec == 0), stop=(ec == 7))
                            nc.scalar.activation(
                                out=qkT[fc][:, 512 * th:512 * th + 512], in_=ps,
                                func=AFT.Identity, bias=qkb[br][:, fc:fc + 1], scale=1.0)
                    # v in standard [t, c] layout, strided by 65 with ones cols
                    for tci in range(8):
                        v3 = vT[tci].rearrange("p (h c) -> p h c", c=65)
                        nc.vector.memset(v3[:, :, 64:65], 1.0)
                        for ch in range(2):
                            ps = psB.tile([128, 512], f32, tag="psB", name="psB")
                            for ec in range(8):
                                nc.tensor.matmul(
                                    ps,
                                    xT_sb[ec][:, 128 * tci:128 * tci + 128],
                                    win_sb[ec][:, 2 * D + 512 * ch:2 * D + 512 * ch + 512],
                                    start=(ec == 0), stop=(ec == 7))
                            nc.vector.tensor_copy(
                                out=v3[:, 8 * ch:8 * ch + 8, 0:64],
                                in_=ps.rearrange("p (h c) -> p h c", c=64))

                attp = brs.enter_context(tc.tile_pool(name=f"att_{br}", bufs=1))
                attnT = [attp.tile([128, T], bf16, tag=f"at{br}{i}", name=f"at{br}{i}") for i in range(8)]

                def normalize(h, ps_av, tp):
                    """ps_av [65, T]: rows 0:64 unnormalized attn, row 64 den.
                    Drain PSUM to SBUF right away (one fast ACT copy) so the
                    bank frees for the next head pair. The reciprocal runs on
                    a DRAM-bounced [128, 8] reshape of the den row: 8 elems
                    per DVE lane (~70ns) instead of 1024 on one lane (5.1us,
                    which would block the strict-FIFO DVE queue and stall the
                    whole pipeline). The recip row then DMA-broadcasts from
                    DRAM into the [64, T] multiplier tile directly."""
                    av_sb = tp.tile([65, T], f32, tag="av_sb", name="av_sb")
                    nc.scalar.copy(out=av_sb, in_=ps_av)
                    d2 = tp.tile([128, 8], f32, tag="d2", name="d2")
                    nc.sync.dma_start(out=d2, in_=av_sb[64:65, :])
                    d2r = tp.tile([128, 8], f32, tag="d2r", name="d2r")
                    nc.vector.reciprocal(out=d2r, in_=d2)
                    dB = dramp.tile([1, T], f32, tag="dB", name="dB")
                    nc.sync.dma_start(
                        out=dB.rearrange("o (p c) -> (o p) c", p=128), in_=d2r)
                    rb = tp.tile([64, T], f32, tag="rb", name="rb")
                    nc.sync.dma_start(out=rb, in_=dB[0:1, :].to_broadcast([64, T]))
                    fc = h // 2
                    vb_col = vb[br][:, h:h + 1]
                    if h % 2 == 0:
                        dst = attnT[fc][0:64, :]
                        nc.vector.tensor_mul(dst, av_sb[0:64, :], rb)
                        nc.vector.tensor_scalar_add(dst, dst, scalar1=vb_col)
                    else:
                        stg = tp.tile([64, T], bf16, tag="stg", name="stg")
                        nc.vector.tensor_mul(stg, av_sb[0:64, :], rb)
                        nc.vector.tensor_scalar_add(stg, stg, scalar1=vb_col)
                        nc.sync.dma_start(out=attnT[fc][64:128, :], in_=stg)

                with tc.tile_pool(name=f"tr_{br}", bufs=2) as tp, \
                     tc.tile_pool(name=f"exp_{br}", bufs=1) as ep, \
                     tc.tile_pool(name=f"psS_{br}", bufs=1, space="PSUM") as psS, \
                     tc.tile_pool(name=f"psV_{br}", bufs=1, space="PSUM") as psV:
                    for hp in range(8):
                        fq, fk = hp, 8 + hp
                        kT = [qkT[fk][0:64, :], qkT[fk][64:128, :]]
                        qT = [qkT[fq][0:64, :], qkT[fq][64:128, :]]
                        if is_local:
                            # both parities' score MMs issued adjacently
                            # (K=64 row-tiles at bases 0/64 run concurrently)
                            # into ONE shared psum tile (parity = 512-col
                            # half) so bufs=2 lets the next half's scores
                            # overlap this half's exp.
                            expT = [ep.tile([128, T], bf16, tag=f"expT{p}",
                                            name=f"expT{p}") for p in range(2)]
                            for half in range(2):
                                ps_s = psS.tile([128, T], f32, tag="ps_sL",
                                                name="ps_sL", bufs=2)
                                for w4 in range(4):
                                    w = 4 * half + w4
                                    sl_w = slice(128 * w, 128 * w + 128)
                                    for p in range(2):
                                        dst = ps_s[:, 512 * p + 128 * w4:
                                                   512 * p + 128 * w4 + 128]
                                        nc.tensor.matmul(dst, kT[p][:, sl_w],
                                                         qT[p][:, sl_w],
                                                         start=True, stop=True)
                                half_sl = slice(512 * half, 512 * half + 512)
                                for p in range(2):
                                    nc.scalar.activation(
                                        out=expT[p][:, half_sl],
                                        in_=ps_s[:, 512 * p:512 * p + 512],
                                        func=AFT.Exp, scale=0.125)
                                    nc.vector.tensor_mul(
                                        expT[p][:, half_sl],
                                        expT[p][:, half_sl],
                                        mask_sb[:, half_sl])
                            ps_av = [psV.tile([65, T], f32, tag=f"ps_av{p}",
                                              name=f"ps_av{p}") for p in range(2)]
                            for w in range(8):
                                sl = slice(128 * w, 128 * w + 128)
                                for p in range(2):
                                    h = 2 * hp + p
                                    nc.tensor.matmul(
                                        ps_av[p][:, sl],
                                        vT[w][:, 65 * h:65 * h + 65],
                                        expT[p][:, sl], start=True, stop=True)
                        else:
                            ps_av = [psV.tile([65, T], f32, tag=f"ps_av{p}",
                                              name=f"ps_av{p}") for p in range(2)]
                            for jc in range(8):
                                ps_s = [psS.tile([128, T], f32, tag=f"ps_s{p}",
                                                 name=f"ps_s{p}") for p in range(2)]
                                for ih in range(2):
                                    sl = slice(512 * ih, 512 * ih + 512)
                                    for p in range(2):
                                        nc.tensor.matmul(
                                            ps_s[p][:, sl],
                                            kT[p][:, 128 * jc:128 * jc + 128],
                                            qT[p][:, sl], start=True, stop=True)
                                b = 128 * jc
                                eTj = []
                                for p in range(2):
                                    eT = ep.tile([128, T], bf16, tag=f"eG{p}",
                                                 name=f"eG{p}", bufs=3)
                                    if jc > 0:
                                        nc.scalar.activation(
                                            out=eT[:, 0:b], in_=ps_s[p][:, 0:b],
                                            func=AFT.Exp, scale=0.125, bias=1.0)
                                    nc.scalar.activation(
                                        out=eT[:, b:T], in_=ps_s[p][:, b:T],
                                        func=AFT.Exp, scale=0.125)
                                    nc.vector.tensor_mul(eT[:, b:b + 128],
                                                         eT[:, b:b + 128],
                                                         mask_sb[:, 0:128])
                                    eTj.append(eT)
                                for ih in range(2):
                                    sl = slice(512 * ih, 512 * ih + 512)
                                    for p in range(2):
                                        h = 2 * hp + p
                                        nc.tensor.matmul(
                                            ps_av[p][:, sl],
                                            vT[jc][:, 65 * h:65 * h + 65],
                                            eTj[p][:, sl],
                                            start=(jc == 0), stop=(jc == 7))
                        normalize(2 * hp, ps_av[0], tp)
                        normalize(2 * hp + 1, ps_av[1], tp)

                # out-projection into fused accumulator
                with tc.tile_pool(name=f"wo_{br}", bufs=1) as wo, \
                     tc.tile_pool(name=f"tmp_{br}", bufs=3) as tq, \
                     tc.tile_pool(name=f"psO_{br}", bufs=4, space="PSUM") as psO:
                    wout_sb = [wo.tile([128, D], bf16, tag=f"wo{br}{i}", name=f"wo{br}{i}")
                               for i in range(8)]
                    for i in range(8):
                        nc.sync.dma_start(out=wout_sb[i],
                                          in_=wout_d[128 * i:128 * i + 128, :])
                    for tci in range(8):
                        for oh in range(2):
                            sl = slice(512 * oh, 512 * oh + 512)
                            ps = psO.tile([128, 512], f32, tag="psO", name="psO")
                            for fc in range(8):
                                nc.tensor.matmul(
                                    ps,
                                    attnT[fc][:, 128 * tci:128 * tci + 128],
                                    wout_sb[fc][:, sl],
                                    start=(fc == 0), stop=(fc == 7))
                            if is_local:
                                nc.vector.tensor_scalar_mul(
                                    facc[tci][:, sl], ps, scalar1=w_sc[br])
                            else:
                                tmp = tq.tile([128, 512], f32, tag="tmp", name="tmp")
                                nc.vector.tensor_scalar_mul(tmp, ps, scalar1=w_sc[br])
                                nc.vector.tensor_add(
                                    facc[tci][:, sl], facc[tci][:, sl], tmp)

        # ---- fuse bias + LayerNorm + output ----
        with tc.tile_pool(name="ln", bufs=3) as lp:
            for tci in range(8):
                f = facc[tci]
                nc.vector.tensor_add(f, f, bcomb)
                stats = lp.tile([128, 2, 6], f32, tag="stats", name="stats")
                for sg in range(2):
                    nc.vector.bn_stats(out=stats[:, sg, :],
                                       in_=f[:, 512 * sg:512 * sg + 512])
                mv = lp.tile([128, 2], f32, tag="mv", name="mv")
                nc.vector.bn_aggr(out=mv, in_=stats)
                sq = lp.tile([128, 1], f32, tag="sq", name="sq")
                nc.scalar.activation(out=sq, in_=mv[:, 1:2], func=AFT.Sqrt,
                                     bias=eps_sb, scale=1.0)
                rstd = lp.tile([128, 1], f32, tag="rstd", name="rstd")
                nc.vector.reciprocal(out=rstd, in_=sq)
                o = lp.tile([128, D], f32, tag="o", name="o")
                nc.vector.tensor_scalar(
                    out=o, in0=f, scalar1=mv[:, 0:1], scalar2=rstd,
                    op0=mybir.AluOpType.subtract, op1=mybir.AluOpType.mult)
                nc.vector.tensor_mul(o, o, gamma_sb)
                nc.vector.tensor_add(o, o, beta_sb)
                nc.sync.dma_start(out=out_d[128 * tci:128 * tci + 128, :], in_=o)
        acc_es.close()

    nc.finalize()
    return nc


def _get_nc():
    if "nc" not in _STATE:
        _STATE["nc"] = _build_nc()
    return _STATE["nc"]


def _host_inputs(inputs):
    """Build the per-core in_maps (host-side sharding/layout/constant-folding)."""
    f = lambda a: np.asarray(a, dtype=np.float32)
    x = f(inputs["x"])
    tri = np.where(np.arange(128)[:, None] > np.arange(128)[None, :],
                   np.float32(_E), np.float32(1.0))
    mask8 = np.tile(tri, (1, 8)).astype(NBF)

    def qk_bias(b_in):
        return np.ascontiguousarray(f(b_in)[:2 * D].reshape(16, 128).T)

    def v_bias(b_in):
        return np.ascontiguousarray(f(b_in)[2 * D:].reshape(16, 64).T)

    # fold the orthogonal projection into the qkv weights (exact algebra:
    # qkv = (x @ P) @ W_in^T = x @ (P @ W_in^T))
    weffT_l = (f(inputs["proj_local"]) @ f(inputs["Wl_in"]).T).astype(NBF)
    weffT_g = (f(inputs["proj_global"]) @ f(inputs["Wg_in"]).T).astype(NBF)

    common = {
        "weffT_l": weffT_l,
        "weffT_g": weffT_g,
        "woutT_l": np.ascontiguousarray(f(inputs["Wl_out"]).T).astype(NBF),
        "woutT_g": np.ascontiguousarray(f(inputs["Wg_out"]).T).astype(NBF),
        "qkb_l": qk_bias(inputs["bl_in"]),
        "qkb_g": qk_bias(inputs["bg_in"]),
        "vb_l": v_bias(inputs["bl_in"]),
        "vb_g": v_bias(inputs["bg_in"]),
        "bout_l": f(inputs["bl_out"]).reshape(1, D),
        "bout_g": f(inputs["bg_out"]).reshape(1, D),
        "fw": f(inputs["fusion_w"]).reshape(1, 2),
        "gamma": f(inputs["ln_gamma"]).reshape(1, D),
        "beta": f(inputs["ln_beta"]).reshape(1, D),
        "mask8": mask8,
    }
    in_maps = []
    for core in range(8):
        b, t0 = core // 4, (core % 4) * T
        xT = np.ascontiguousarray(x[b, t0:t0 + T, :].T).astype(NBF)
        in_maps.append({**common, "xT": xT})
    return in_maps


def _run(inputs, trace=False):
    from concourse.bass_utils import run_bass_kernel_spmd

    nc = _get_nc()
    in_maps = _host_inputs(inputs)
    res = run_bass_kernel_spmd(nc, in_maps, core_ids=list(range(8)), trace=trace)
    x = np.asarray(inputs["x"])
    out = np.empty((2, 4096, D), np.float32)
    for core in range(8):
        b, t0 = core // 4, (core % 4) * T
        out[b, t0:t0 + T, :] = res.results[core]["out"]
    return out, res


def kernel(**inputs) -> np.ndarray:
    out, _ = _run(inputs)
    return out


# revision 10
# speedup vs baseline: 1.2252x; 1.0017x over previous
"""Trainium2 Bass kernel for nn_HAOQAttention (hierarchical attention with
orthogonal query decomposition), data-parallel over 8 NeuronCores.

Sharding: x is [2, 4096, 1024]; each core takes one contiguous 1024-token
slice (= exactly one GLOBAL_W=1024 window = 8 LOCAL_W=128 windows), so the
whole computation is embarrassingly parallel across cores with replicated
weights. All matmuls run in bf16 on the PE; softmax statistics, layernorm
and accumulations stay in fp32 (PSUM).

The orthogonal projections are folded into the QKV weights on the host
(W_eff = P @ W_in^T, exact algebra), so the device pipeline is:
  xT [d, t] -> qkT [f, t] (features on partitions), v in standard [t, c]
  layout with an interleaved ones-column so the AV matmul emits the softmax
  denominator as PSUM row 64 for free.
  scoresT [j, i] = kT^T @ qT with the two heads of each 128-feature chunk
  issued back-to-back at partition bases 0/64 (PE row-tile concurrency for
  the K=64 contractions); exp via ACT with the additive mask folded in
  (constant +1 bias for fully-strictly-lower column ranges, a {1,e}
  multiplicative triangle for diagonal blocks).
  attnT [din, t] feeds the output projection directly; final LN is done in
  standard [t, d] layout where all reductions are free-dim native.
"""

import sys

sys.path.insert(0, "/opt/trn_rl_repo")

import numpy as np
import ml_dtypes

NBF = ml_dtypes.bfloat16

D = 1024
T = 1024            # tokens per core
_E = float(np.e)

_STATE = {}


def _build_nc():
    from contextlib import ExitStack

    import concourse.mybir as mybir
    from concourse import bacc
    from concourse.tile import TileContext

    bf16 = mybir.dt.bfloat16
    f32 = mybir.dt.float32
    AFT = mybir.ActivationFunctionType

    nc = bacc.Bacc(None, target_bir_lowering=False)

    P = nc.declare_dram_parameter
    xT_d = P("xT", [D, T], bf16, isOutput=False)
    winl_d = P("weffT_l", [D, 3 * D], bf16, isOutput=False)
    wing_d = P("weffT_g", [D, 3 * D], bf16, isOutput=False)
    woutl_d = P("woutT_l", [D, D], bf16, isOutput=False)
    woutg_d = P("woutT_g", [D, D], bf16, isOutput=False)
    qkbl_d = P("qkb_l", [128, 16], f32, isOutput=False)
    qkbg_d = P("qkb_g", [128, 16], f32, isOutput=False)
    vbl_d = P("vb_l", [64, 16], f32, isOutput=False)
    vbg_d = P("vb_g", [64, 16], f32, isOutput=False)
    boutl_d = P("bout_l", [1, D], f32, isOutput=False)
    boutg_d = P("bout_g", [1, D], f32, isOutput=False)
    fw_d = P("fw", [1, 2], f32, isOutput=False)
    gamma_d = P("gamma", [1, D], f32, isOutput=False)
    beta_d = P("beta", [1, D], f32, isOutput=False)
    mask8_d = P("mask8", [128, T], bf16, isOutput=False)
    out_d = P("out", [T, D], f32, isOutput=True)

    with TileContext(nc) as tc, ExitStack() as top:
        const = top.enter_context(tc.tile_pool(name="const", bufs=1))

        mask_sb = const.tile([128, T], bf16)
        nc.sync.dma_start(out=mask_sb, in_=mask8_d[:, :])
        gamma_sb = const.tile([128, D], f32)
        nc.sync.dma_start(out=gamma_sb, in_=gamma_d[0:1, :].to_broadcast([128, D]))
        beta_sb = const.tile([128, D], f32)
        nc.sync.dma_start(out=beta_sb, in_=beta_d[0:1, :].to_broadcast([128, D]))
        qkb = {}
        vb = {}
        for br, (qd, vd) in {"l": (qkbl_d, vbl_d), "g": (qkbg_d, vbg_d)}.items():
            qkb[br] = const.tile([128, 16], f32, tag=f"qkb_{br}", name=f"qkb_{br}")
            nc.sync.dma_start(out=qkb[br], in_=qd[:, :])
            vb[br] = const.tile([64, 16], f32, tag=f"vb_{br}", name=f"vb_{br}")
            nc.sync.dma_start(out=vb[br], in_=vd[:, :])

        # fusion softmax weights (2 entries) computed on partition 0
        fw_sb = const.tile([1, 2], f32)
        nc.sync.dma_start(out=fw_sb, in_=fw_d[:, :])
        fe = const.tile([1, 2], f32)
        nc.scalar.activation(out=fe, in_=fw_sb, func=AFT.Exp)
        fs = const.tile([1, 1], f32)
        nc.vector.tensor_add(fs, fe[:, 0:1], fe[:, 1:2])
        fr = const.tile([1, 1], f32)
        nc.vector.reciprocal(out=fr, in_=fs)
        fwn = const.tile([1, 2], f32)
        nc.vector.tensor_scalar_mul(fwn, fe, scalar1=fr)
        fwb = const.tile([128, 2], f32)
        nc.gpsimd.partition_broadcast(fwb, fwn)
        w_sc = {"l": fwb[:, 0:1], "g": fwb[:, 1:2]}

        # combined output-projection bias: w0*b_out_l + w1*b_out_g, broadcast
        bl_sb = const.tile([1, D], f32)
        nc.sync.dma_start(out=bl_sb, in_=boutl_d[:, :])
        bg_sb = const.tile([1, D], f32)
        nc.sync.dma_start(out=bg_sb, in_=boutg_d[:, :])
        bt0 = const.tile([1, D], f32)
        nc.vector.tensor_scalar_mul(bt0, bl_sb, scalar1=fwn[0:1, 0:1])
        bt1 = const.tile([1, D], f32)
        nc.vector.tensor_scalar_mul(bt1, bg_sb, scalar1=fwn[0:1, 1:2])
        bc0 = const.tile([1, D], f32)
        nc.vector.tensor_add(bc0, bt0, bt1)
        bcomb = const.tile([128, D], f32)
        nc.gpsimd.partition_broadcast(bcomb, bc0)
        eps_sb = const.tile([128, 1], f32)
        nc.vector.memset(eps_sb, 1e-5)

        # DRAM scratch for the per-head denominator reshape bounce
        dramp = top.enter_context(tc.tile_pool(name="dram", bufs=2, space="DRAM"))

        # input activations, resident for both branches
        xtp = top.enter_context(tc.tile_pool(name="xtp", bufs=1))
        xT_sb = [xtp.tile([128, T], bf16, tag=f"xT{i}", name=f"xT{i}") for i in range(8)]
        for i in range(8):
            nc.sync.dma_start(out=xT_sb[i], in_=xT_d[128 * i:128 * i + 128, :])

        acc_es = ExitStack()
        accp = acc_es.enter_context(tc.tile_pool(name="acc", bufs=1))
        facc = [accp.tile([128, D], f32, tag=f"acc{i}", name=f"acc{i}") for i in range(8)]

        # ---- per-branch: qkv -> windowed attention -> out-proj ----
        for br, win_d, wout_d, is_local in (
            ("l", winl_d, woutl_d, True),
            ("g", wing_d, woutg_d, False),
        ):
            with ExitStack() as brs:
                qkvp = brs.enter_context(tc.tile_pool(name=f"qkv_{br}", bufs=1))
                qkT = [qkvp.tile([128, T], bf16, tag=f"qk{br}{i}", name=f"qk{br}{i}") for i in range(16)]
                vT = [qkvp.tile([128, 16 * 65], bf16, tag=f"v{br}{i}", name=f"v{br}{i}") for i in range(8)]

                with tc.tile_pool(name=f"w_{br}", bufs=1) as wp, \
                     tc.tile_pool(name=f"psB_{br}", bufs=4, space="PSUM") as psB:
                    win_sb = [wp.tile([128, 3 * D], bf16, tag=f"win{br}{i}", name=f"win{br}{i}")
                              for i in range(8)]
                    for i in range(8):
                        nc.sync.dma_start(out=win_sb[i],
                                          in_=win_d[128 * i:128 * i + 128, :])
                    # q,k chunks: qkT[fc] = (W_eff[:, fc])^T @ xT, plus bias
                    for fc in range(16):
                        for th in range(2):
                            ps = psB.tile([128, 512], f32, tag="psB", name="psB")
                            for ec in range(8):
                                nc.tensor.matmul(
                                    ps,
                                    win_sb[ec][:, 128 * fc:128 * fc + 128],
                                    xT_sb[ec][:, 512 * th:512 * th + 512],
                                    start=(ec == 0), stop=(ec == 7))
                            nc.scalar.activation(
                                out=qkT[fc][:, 512 * th:512 * th + 512], in_=ps,
                                func=AFT.Identity, bias=qkb[br][:, fc:fc + 1], scale=1.0)
                    # v in standard [t, c] layout, strided by 65 with ones cols
                    for tci in range(8):
                        v3 = vT[tci].rearrange("p (h c) -> p h c", c=65)
                        nc.vector.memset(v3[:, :, 64:65], 1.0)
                        for ch in range(2):
                            ps = psB.tile([128, 512], f32, tag="psB", name="psB")
                            for ec in range(8):
                                nc.tensor.matmul(
                                    ps,
                                    xT_sb[ec][:, 128 * tci:128 * tci + 128],
                                    win_sb[ec][:, 2 * D + 512 * ch:2 * D + 512 * ch + 512],
                                    start=(ec == 0), stop=(ec == 7))
                            nc.vector.tensor_copy(
                                out=v3[:, 8 * ch:8 * ch + 8, 0:64],
                                in_=ps.rearrange("p (h c) -> p h c", c=64))

                attp = brs.enter_context(tc.tile_pool(name=f"att_{br}", bufs=1))
                attnT = [attp.tile([128, T], bf16, tag=f"at{br}{i}", name=f"at{br}{i}") for i in range(8)]

                def normalize(h, ps_av, tp):
                    """ps_av [65, T]: rows 0:64 unnormalized attn, row 64 den.
                    Drain PSUM to SBUF right away (one fast ACT copy) so the
                    bank frees for the next head pair. The reciprocal runs on
                    a DRAM-bounced [128, 8] reshape of the den row: 8 elems
                    per DVE lane (~70ns) instead of 1024 on one lane (5.1us,
                    which would block the strict-FIFO DVE queue and stall the
                    whole pipeline). The recip row then DMA-broadcasts from
                    DRAM into the [64, T] multiplier tile directly."""
                    av_sb = tp.tile([65, T], f32, tag="av_sb", name="av_sb")
                    nc.vector.tensor_copy(out=av_sb, in_=ps_av)
                    d2 = tp.tile([128, 8], f32, tag="d2", name="d2")
                    nc.sync.dma_start(out=d2, in_=av_sb[64:65, :])
                    d2r = tp.tile([128, 8], f32, tag="d2r", name="d2r")
                    nc.vector.reciprocal(out=d2r, in_=d2)
                    dB = dramp.tile([1, T], f32, tag="dB", name="dB")
                    nc.sync.dma_start(
                        out=dB.rearrange("o (p c) -> (o p) c", p=128), in_=d2r)
                    rb = tp.tile([64, T], f32, tag="rb", name="rb")
                    nc.sync.dma_start(out=rb, in_=dB[0:1, :].to_broadcast([64, T]))
                    fc = h // 2
                    vb_col = vb[br][:, h:h + 1]
                    if h % 2 == 0:
                        dst = attnT[fc][0:64, :]
                        nc.vector.tensor_mul(dst, av_sb[0:64, :], rb)
                        nc.vector.tensor_scalar_add(dst, dst, scalar1=vb_col)
                    else:
                        stg = tp.tile([64, T], bf16, tag="stg", name="stg")
                        nc.vector.tensor_mul(stg, av_sb[0:64, :], rb)
                        nc.vector.tensor_scalar_add(stg, stg, scalar1=vb_col)
                        nc.sync.dma_start(out=attnT[fc][64:128, :], in_=stg)

                with tc.tile_pool(name=f"tr_{br}", bufs=2) as tp, \
                     tc.tile_pool(name=f"exp_{br}", bufs=1) as ep, \
                     tc.tile_pool(name=f"psS_{br}", bufs=1, space="PSUM") as psS, \
                     tc.tile_pool(name=f"psV_{br}", bufs=1, space="PSUM") as psV:
                    for hp in range(8):
                        fq, fk = hp, 8 + hp
                        kT = [qkT[fk][0:64, :], qkT[fk][64:128, :]]
                        qT = [qkT[fq][0:64, :], qkT[fq][64:128, :]]
                        if is_local:
                            # both parities' score MMs issued adjacently
                            # (K=64 row-tiles at bases 0/64 run concurrently)
                            # into ONE shared psum tile (parity = 512-col
                            # half) so bufs=2 lets the next half's scores
                            # overlap this half's exp.
                            expT = [ep.tile([128, T], bf16, tag=f"expT{p}",
                                            name=f"expT{p}") for p in range(2)]
                            for half in range(2):
                                ps_s = psS.tile([128, T], f32, tag="ps_sL",
                                                name="ps_sL", bufs=2)
                                for w4 in range(4):
                                    w = 4 * half + w4
                                    sl_w = slice(128 * w, 128 * w + 128)
                                    for p in range(2):
                                        dst = ps_s[:, 512 * p + 128 * w4:
                                                   512 * p + 128 * w4 + 128]
                                        nc.tensor.matmul(dst, kT[p][:, sl_w],
                                                         qT[p][:, sl_w],
                                                         start=True, stop=True)
                                half_sl = slice(512 * half, 512 * half + 512)
                                for p in range(2):
                                    nc.scalar.activation(
                                        out=expT[p][:, half_sl],
                                        in_=ps_s[:, 512 * p:512 * p + 512],
                                        func=AFT.Exp, scale=0.125)
                                    nc.vector.tensor_mul(
                                        expT[p][:, half_sl],
                                        expT[p][:, half_sl],
                                        mask_sb[:, half_sl])
                            ps_av = [psV.tile([65, T], f32, tag=f"ps_av{p}",
                                              name=f"ps_av{p}") for p in range(2)]
                            for w in range(8):
                                sl = slice(128 * w, 128 * w + 128)
                                for p in range(2):
                                    h = 2 * hp + p
                                    nc.tensor.matmul(
                                        ps_av[p][:, sl],
                                        vT[w][:, 65 * h:65 * h + 65],
                                        expT[p][:, sl], start=True, stop=True)
                        else:
                            ps_av = [psV.tile([65, T], f32, tag=f"ps_av{p}",
                                              name=f"ps_av{p}") for p in range(2)]
                            for jc in range(8):
                                b = 128 * jc
                                eTj = [ep.tile([128, T], bf16, tag=f"eG{p}",
                                               name=f"eG{p}", bufs=3)
                                       for p in range(2)]
                                for ih in range(2):
                                    lo, hi = 512 * ih, 512 * ih + 512
                                    for p in range(2):
                                        ps_s = psS.tile([128, 512], f32,
                                                        tag=f"ps_s{p}",
                                                        name=f"ps_s{p}", bufs=2)
                                        nc.tensor.matmul(
                                            ps_s,
                                            kT[p][:, 128 * jc:128 * jc + 128],
                                            qT[p][:, lo:hi],
                                            start=True, stop=True)
                                        eT = eTj[p]
                                        # exp with the additive causal(+1) mask
                                        # folded in, per 512-col half:
                                        # cols < b get bias 1, cols >= b+128
                                        # get bias 0, the diagonal 128-block
                                        # gets bias 0 + {1,e} triangle mul.
                                        if b >= hi:
                                            nc.scalar.activation(
                                                out=eT[:, lo:hi], in_=ps_s,
                                                func=AFT.Exp, scale=0.125,
                                                bias=1.0)
                                        elif b + 128 <= lo:
                                            nc.scalar.activation(
                                                out=eT[:, lo:hi], in_=ps_s,
                                                func=AFT.Exp, scale=0.125)
                                        else:
                                            if b > lo:
                                                nc.scalar.activation(
                                                    out=eT[:, lo:b],
                                                    in_=ps_s[:, 0:b - lo],
                                                    func=AFT.Exp, scale=0.125,
                                                    bias=1.0)
                                            nc.scalar.activation(
                                                out=eT[:, b:hi],
                                                in_=ps_s[:, b - lo:512],
                                                func=AFT.Exp, scale=0.125)
                                            nc.vector.tensor_mul(
                                                eT[:, b:b + 128],
                                                eT[:, b:b + 128],
                                                mask_sb[:, 0:128])
                                for ih in range(2):
                                    sl = slice(512 * ih, 512 * ih + 512)
                                    for p in range(2):
                                        h = 2 * hp + p
                                        nc.tensor.matmul(
                                            ps_av[p][:, sl],
                                            vT[jc][:, 65 * h:65 * h + 65],
                                            eTj[p][:, sl],
                                            start=(jc == 0), stop=(jc == 7))
                        normalize(2 * hp, ps_av[0], tp)
                        normalize(2 * hp + 1, ps_av[1], tp)

                # out-projection into fused accumulator
                with tc.tile_pool(name=f"wo_{br}", bufs=1) as wo, \
                     tc.tile_pool(name=f"tmp_{br}", bufs=3) as tq, \
                     tc.tile_pool(name=f"psO_{br}", bufs=4, space="PSUM") as psO:
                    wout_sb = [wo.tile([128, D], bf16, tag=f"wo{br}{i}", name=f"wo{br}{i}")
                               for i in range(8)]
                    for i in range(8):
                        nc.sync.dma_start(out=wout_sb[i],
                                          in_=wout_d[128 * i:128 * i + 128, :])
                    for tci in range(8):
                        for oh in range(2):
                            sl = slice(512 * oh, 512 * oh + 512)
                            ps = psO.tile([128, 512], f32, tag="psO", name="psO")
                            for fc in range(8):
                                nc.tensor.matmul(
                                    ps,
                                    attnT[fc][:, 128 * tci:128 * tci + 128],
                                    wout_sb[fc][:, sl],
                                    start=(fc == 0), stop=(fc == 7))
                            if is_local:
                                nc.vector.tensor_scalar_mul(
                                    facc[tci][:, sl], ps, scalar1=w_sc[br])
                            else:
                                tmp = tq.tile([128, 512], f32, tag="tmp", name="tmp")
                                nc.vector.tensor_scalar_mul(tmp, ps, scalar1=w_sc[br])
                                nc.vector.tensor_add(
                                    facc[tci][:, sl], facc[tci][:, sl], tmp)

        # ---- fuse bias + LayerNorm + output ----
        with tc.tile_pool(name="ln", bufs=3) as lp:
            for tci in range(8):
                f = facc[tci]
                nc.vector.tensor_add(f, f, bcomb)
                stats = lp.tile([128, 2, 6], f32, tag="stats", name="stats")
                for sg in range(2):
                    nc.vector.bn_stats(out=stats[:, sg, :],
                                       in_=f[:, 512 * sg:512 * sg + 512])
                mv = lp.tile([128, 2], f32, tag="mv", name="mv")
                nc.vector.bn_aggr(out=mv, in_=stats)
                sq = lp.tile([128, 1], f32, tag="sq", name="sq")
                nc.scalar.activation(out=sq, in_=mv[:, 1:2], func=AFT.Sqrt,
                                     bias=eps_sb, scale=1.0)
                rstd = lp.tile([128, 1], f32, tag="rstd", name="rstd")
                nc.vector.reciprocal(out=rstd, in_=sq)
                o = lp.tile([128, D], f32, tag="o", name="o")
                nc.vector.tensor_scalar(
                    out=o, in0=f, scalar1=mv[:, 0:1], scalar2=rstd,
                    op0=mybir.AluOpType.subtract, op1=mybir.AluOpType.mult)
                nc.vector.tensor_mul(o, o, gamma_sb)
                nc.vector.tensor_add(o, o, beta_sb)
                nc.sync.dma_start(out=out_d[128 * tci:128 * tci + 128, :], in_=o)
        acc_es.close()

    nc.finalize()
    return nc


def _get_nc():
    if "nc" not in _STATE:
        _STATE["nc"] = _build_nc()
    return _STATE["nc"]


def _host_inputs(inputs):
    """Build the per-core in_maps (host-side sharding/layout/constant-folding)."""
    f = lambda a: np.asarray(a, dtype=np.float32)
    x = f(inputs["x"])
    tri = np.where(np.arange(128)[:, None] > np.arange(128)[None, :],
                   np.float32(_E), np.float32(1.0))
    mask8 = np.tile(tri, (1, 8)).astype(NBF)

    def qk_bias(b_in):
        return np.ascontiguousarray(f(b_in)[:2 * D].reshape(16, 128).T)

    def v_bias(b_in):
        return np.ascontiguousarray(f(b_in)[2 * D:].reshape(16, 64).T)

    # fold the orthogonal projection into the qkv weights (exact algebra:
    # qkv = (x @ P) @ W_in^T = x @ (P @ W_in^T))
    weffT_l = (f(inputs["proj_local"]) @ f(inputs["Wl_in"]).T).astype(NBF)
    weffT_g = (f(inputs["proj_global"]) @ f(inputs["Wg_in"]).T).astype(NBF)

    common = {
        "weffT_l": weffT_l,
        "weffT_g": weffT_g,
        "woutT_l": np.ascontiguousarray(f(inputs["Wl_out"]).T).astype(NBF),
        "woutT_g": np.ascontiguousarray(f(inputs["Wg_out"]).T).astype(NBF),
        "qkb_l": qk_bias(inputs["bl_in"]),
        "qkb_g": qk_bias(inputs["bg_in"]),
        "vb_l": v_bias(inputs["bl_in"]),
        "vb_g": v_bias(inputs["bg_in"]),
        "bout_l": f(inputs["bl_out"]).reshape(1, D),
        "bout_g": f(inputs["bg_out"]).reshape(1, D),
        "fw": f(inputs["fusion_w"]).reshape(1, 2),
        "gamma": f(inputs["ln_gamma"]).reshape(1, D),
        "beta": f(inputs["ln_beta"]).reshape(1, D),
        "mask8": mask8,
    }
    in_maps = []
    for core in range(8):
        b, t0 = core // 4, (core % 4) * T
        xT = np.ascontiguousarray(x[b, t0:t0 + T, :].T).astype(NBF)
        in_maps.append({**common, "xT": xT})
    return in_maps


def _run(inputs, trace=False):
    from concourse.bass_utils import run_bass_kernel_spmd

    nc = _get_nc()
    in_maps = _host_inputs(inputs)
    res = run_bass_kernel_spmd(nc, in_maps, core_ids=list(range(8)), trace=trace)
    x = np.asarray(inputs["x"])
    out = np.empty((2, 4096, D), np.float32)
    for core in range(8):
        b, t0 = core // 4, (core % 4) * T
        out[b, t0:t0 + T, :] = res.results[core]["out"]
    return out, res


def kernel(**inputs) -> np.ndarray:
    out, _ = _run(inputs)
    return out


# revision 11
# speedup vs baseline: 1.2881x; 1.0513x over previous
"""Trainium2 Bass kernel for nn_HAOQAttention (hierarchical attention with
orthogonal query decomposition), data-parallel over 8 NeuronCores.

Sharding: x is [2, 4096, 1024]; each core takes one contiguous 1024-token
slice (= exactly one GLOBAL_W=1024 window = 8 LOCAL_W=128 windows), so the
whole computation is embarrassingly parallel across cores with replicated
weights. All matmuls run in bf16 on the PE; softmax statistics, layernorm
and accumulations stay in fp32 (PSUM).

The orthogonal projections are folded into the QKV weights on the host
(W_eff = P @ W_in^T, exact algebra), so the device pipeline is:
  xT [d, t] -> qkT [f, t] (features on partitions), v in standard [t, c]
  layout with an interleaved ones-column so the AV matmul emits the softmax
  denominator as PSUM row 64 for free.
  scoresT [j, i] = kT^T @ qT with the two heads of each 128-feature chunk
  issued back-to-back at partition bases 0/64 (PE row-tile concurrency for
  the K=64 contractions); exp via ACT with the additive mask folded in
  (constant +1 bias for fully-strictly-lower column ranges, a {1,e}
  multiplicative triangle for diagonal blocks).
  attnT [din, t] feeds the output projection directly; final LN is done in
  standard [t, d] layout where all reductions are free-dim native.
"""

import sys

sys.path.insert(0, "/opt/trn_rl_repo")

import numpy as np
import ml_dtypes

NBF = ml_dtypes.bfloat16

D = 1024
T = 1024            # tokens per core
_E = float(np.e)

_STATE = {}


def _build_nc():
    from contextlib import ExitStack

    import concourse.mybir as mybir
    from concourse import bacc
    from concourse.tile import TileContext

    bf16 = mybir.dt.bfloat16
    f32 = mybir.dt.float32
    AFT = mybir.ActivationFunctionType

    nc = bacc.Bacc(None, target_bir_lowering=False)

    P = nc.declare_dram_parameter
    xT_d = P("xT", [D, T], bf16, isOutput=False)
    winl_d = P("weffT_l", [D, 3 * D], bf16, isOutput=False)
    wing_d = P("weffT_g", [D, 3 * D], bf16, isOutput=False)
    woutl_d = P("woutT_l", [D, D], bf16, isOutput=False)
    woutg_d = P("woutT_g", [D, D], bf16, isOutput=False)
    qkbl_d = P("qkb_l", [128, 16], f32, isOutput=False)
    qkbg_d = P("qkb_g", [128, 16], f32, isOutput=False)
    vbl_d = P("vb_l", [64, 16], f32, isOutput=False)
    vbg_d = P("vb_g", [64, 16], f32, isOutput=False)
    boutl_d = P("bout_l", [1, D], f32, isOutput=False)
    boutg_d = P("bout_g", [1, D], f32, isOutput=False)
    fw_d = P("fw", [1, 2], f32, isOutput=False)
    gamma_d = P("gamma", [1, D], f32, isOutput=False)
    beta_d = P("beta", [1, D], f32, isOutput=False)
    mask8_d = P("mask8", [128, T], bf16, isOutput=False)
    out_d = P("out", [T, D], f32, isOutput=True)

    with TileContext(nc) as tc, ExitStack() as top:
        const = top.enter_context(tc.tile_pool(name="const", bufs=1))

        mask_sb = const.tile([128, T], bf16)
        nc.sync.dma_start(out=mask_sb, in_=mask8_d[:, :])
        gamma_sb = const.tile([128, D], f32)
        nc.sync.dma_start(out=gamma_sb, in_=gamma_d[0:1, :].to_broadcast([128, D]))
        beta_sb = const.tile([128, D], f32)
        nc.sync.dma_start(out=beta_sb, in_=beta_d[0:1, :].to_broadcast([128, D]))
        qkb = {}
        vb = {}
        for br, (qd, vd) in {"l": (qkbl_d, vbl_d), "g": (qkbg_d, vbg_d)}.items():
            qkb[br] = const.tile([128, 16], f32, tag=f"qkb_{br}", name=f"qkb_{br}")
            nc.sync.dma_start(out=qkb[br], in_=qd[:, :])
            vb[br] = const.tile([64, 16], f32, tag=f"vb_{br}", name=f"vb_{br}")
            nc.sync.dma_start(out=vb[br], in_=vd[:, :])

        # fusion softmax weights (2 entries) computed on partition 0
        fw_sb = const.tile([1, 2], f32)
        nc.sync.dma_start(out=fw_sb, in_=fw_d[:, :])
        fe = const.tile([1, 2], f32)
        nc.scalar.activation(out=fe, in_=fw_sb, func=AFT.Exp)
        fs = const.tile([1, 1], f32)
        nc.vector.tensor_add(fs, fe[:, 0:1], fe[:, 1:2])
        fr = const.tile([1, 1], f32)
        nc.vector.reciprocal(out=fr, in_=fs)
        fwn = const.tile([1, 2], f32)
        nc.vector.tensor_scalar_mul(fwn, fe, scalar1=fr)
        fwb = const.tile([128, 2], f32)
        nc.gpsimd.partition_broadcast(fwb, fwn)
        w_sc = {"l": fwb[:, 0:1], "g": fwb[:, 1:2]}

        # combined output-projection bias: w0*b_out_l + w1*b_out_g, broadcast
        bl_sb = const.tile([1, D], f32)
        nc.sync.dma_start(out=bl_sb, in_=boutl_d[:, :])
        bg_sb = const.tile([1, D], f32)
        nc.sync.dma_start(out=bg_sb, in_=boutg_d[:, :])
        bt0 = const.tile([1, D], f32)
        nc.vector.tensor_scalar_mul(bt0, bl_sb, scalar1=fwn[0:1, 0:1])
        bt1 = const.tile([1, D], f32)
        nc.vector.tensor_scalar_mul(bt1, bg_sb, scalar1=fwn[0:1, 1:2])
        bc0 = const.tile([1, D], f32)
        nc.vector.tensor_add(bc0, bt0, bt1)
        bcomb = const.tile([128, D], f32)
        nc.gpsimd.partition_broadcast(bcomb, bc0)
        eps_sb = const.tile([128, 1], f32)
        nc.vector.memset(eps_sb, 1e-5)

        # DRAM scratch for the per-head denominator reshape bounce
        dramp = top.enter_context(tc.tile_pool(name="dram", bufs=2, space="DRAM"))

        # input activations, resident for both branches
        xtp = top.enter_context(tc.tile_pool(name="xtp", bufs=1))
        xT_sb = [xtp.tile([128, T], bf16, tag=f"xT{i}", name=f"xT{i}") for i in range(8)]
        for i in range(8):
            nc.sync.dma_start(out=xT_sb[i], in_=xT_d[128 * i:128 * i + 128, :])

        acc_es = ExitStack()
        accp = acc_es.enter_context(tc.tile_pool(name="acc", bufs=1))
        facc = [accp.tile([128, D], f32, tag=f"acc{i}", name=f"acc{i}") for i in range(8)]

        # ---- per-branch: qkv -> windowed attention -> out-proj ----
        for br, win_d, wout_d, is_local in (
            ("l", winl_d, woutl_d, True),
            ("g", wing_d, woutg_d, False),
        ):
            with ExitStack() as brs:
                qkvp = brs.enter_context(tc.tile_pool(name=f"qkv_{br}", bufs=1))
                qkT = [qkvp.tile([128, T], bf16, tag=f"qk{br}{i}", name=f"qk{br}{i}") for i in range(16)]
                vT = [qkvp.tile([128, 16 * 65], bf16, tag=f"v{br}{i}", name=f"v{br}{i}") for i in range(8)]

                with tc.tile_pool(name=f"w_{br}", bufs=1) as wp, \
                     tc.tile_pool(name=f"psB_{br}", bufs=4, space="PSUM") as psB:
                    win_sb = [wp.tile([128, 3 * D], bf16, tag=f"win{br}{i}", name=f"win{br}{i}")
                              for i in range(8)]
                    for cch in range(4):
                        csl = slice(768 * cch, 768 * cch + 768)
                        for i in range(8):
                            nc.sync.dma_start(out=win_sb[i][:, csl],
                                              in_=win_d[128 * i:128 * i + 128, csl])
                    # q,k chunks: qkT[fc] = (W_eff[:, fc])^T @ xT, plus bias
                    for fc in range(16):
                        for th in range(2):
                            ps = psB.tile([128, 512], f32, tag="psB", name="psB")
                            for ec in range(8):
                                nc.tensor.matmul(
                                    ps,
                                    win_sb[ec][:, 128 * fc:128 * fc + 128],
                                    xT_sb[ec][:, 512 * th:512 * th + 512],
                                    start=(ec == 0), stop=(ec == 7))
                            nc.scalar.activation(
                                out=qkT[fc][:, 512 * th:512 * th + 512], in_=ps,
                                func=AFT.Identity, bias=qkb[br][:, fc:fc + 1], scale=1.0)
                    # v in standard [t, c] layout, strided by 65 with ones cols
                    for tci in range(8):
                        v3 = vT[tci].rearrange("p (h c) -> p h c", c=65)
                        nc.vector.memset(v3[:, :, 64:65], 1.0)
                        for ch in range(2):
                            ps = psB.tile([128, 512], f32, tag="psB", name="psB")
                            for ec in range(8):
                                nc.tensor.matmul(
                                    ps,
                                    xT_sb[ec][:, 128 * tci:128 * tci + 128],
                                    win_sb[ec][:, 2 * D + 512 * ch:2 * D + 512 * ch + 512],
                                    start=(ec == 0), stop=(ec == 7))
                            nc.vector.tensor_copy(
                                out=v3[:, 8 * ch:8 * ch + 8, 0:64],
                                in_=ps.rearrange("p (h c) -> p h c", c=64))

                attp = brs.enter_context(tc.tile_pool(name=f"att_{br}", bufs=1))
                attnT = [attp.tile([128, T], bf16, tag=f"at{br}{i}", name=f"at{br}{i}") for i in range(8)]

                def normalize(h, ps_av, tp):
                    """ps_av [65, T]: rows 0:64 unnormalized attn, row 64 den.
                    Drain PSUM to SBUF right away (one fast ACT copy) so the
                    bank frees for the next head pair. The reciprocal runs on
                    a DRAM-bounced [128, 8] reshape of the den row: 8 elems
                    per DVE lane (~70ns) instead of 1024 on one lane (5.1us,
                    which would block the strict-FIFO DVE queue and stall the
                    whole pipeline). The recip row then DMA-broadcasts from
                    DRAM into the [64, T] multiplier tile directly."""
                    av_sb = tp.tile([65, T], f32, tag="av_sb", name="av_sb")
                    nc.vector.tensor_copy(out=av_sb, in_=ps_av)
                    d2 = tp.tile([128, 8], f32, tag="d2", name="d2")
                    nc.sync.dma_start(out=d2, in_=av_sb[64:65, :])
                    d2r = tp.tile([128, 8], f32, tag="d2r", name="d2r")
                    nc.vector.reciprocal(out=d2r, in_=d2)
                    dB = dramp.tile([1, T], f32, tag="dB", name="dB")
                    nc.sync.dma_start(
                        out=dB.rearrange("o (p c) -> (o p) c", p=128), in_=d2r)
                    rb = tp.tile([64, T], f32, tag="rb", name="rb")
                    nc.sync.dma_start(out=rb, in_=dB[0:1, :].to_broadcast([64, T]))
                    fc = h // 2
                    vb_col = vb[br][:, h:h + 1]
                    if h % 2 == 0:
                        dst = attnT[fc][0:64, :]
                        nc.vector.tensor_mul(dst, av_sb[0:64, :], rb)
                        nc.vector.tensor_scalar_add(dst, dst, scalar1=vb_col)
                    else:
                        stg = tp.tile([64, T], bf16, tag="stg", name="stg")
                        nc.vector.tensor_mul(stg, av_sb[0:64, :], rb)
                        nc.vector.tensor_scalar_add(stg, stg, scalar1=vb_col)
                        nc.sync.dma_start(out=attnT[fc][64:128, :], in_=stg)

                wo_es = ExitStack()
                wo = wo_es.enter_context(tc.tile_pool(name=f"wo_{br}", bufs=1))
                wout_sb = [wo.tile([128, D], bf16, tag=f"wo{br}{i}", name=f"wo{br}{i}")
                           for i in range(8)]
                for i in range(8):
                    nc.sync.dma_start(out=wout_sb[i],
                                      in_=wout_d[128 * i:128 * i + 128, :])

                with tc.tile_pool(name=f"tr_{br}", bufs=2) as tp, \
                     tc.tile_pool(name=f"exp_{br}", bufs=1) as ep, \
                     tc.tile_pool(name=f"psS_{br}", bufs=1, space="PSUM") as psS, \
                     tc.tile_pool(name=f"psV_{br}", bufs=1, space="PSUM") as psV:
                    for hp in range(8):
                        fq, fk = hp, 8 + hp
                        kT = [qkT[fk][0:64, :], qkT[fk][64:128, :]]
                        qT = [qkT[fq][0:64, :], qkT[fq][64:128, :]]
                        if is_local:
                            # both parities' score MMs issued adjacently
                            # (K=64 row-tiles at bases 0/64 run concurrently)
                            # into ONE shared psum tile (parity = 512-col
                            # half) so bufs=2 lets the next half's scores
                            # overlap this half's exp.
                            expT = [ep.tile([128, T], bf16, tag=f"expT{p}",
                                            name=f"expT{p}", bufs=2)
                                    for p in range(2)]
                            for half in range(2):
                                ps_s = psS.tile([128, T], f32, tag="ps_sL",
                                                name="ps_sL", bufs=2)
                                for w4 in range(4):
                                    w = 4 * half + w4
                                    sl_w = slice(128 * w, 128 * w + 128)
                                    for p in range(2):
                                        dst = ps_s[:, 512 * p + 128 * w4:
                                                   512 * p + 128 * w4 + 128]
                                        nc.tensor.matmul(dst, kT[p][:, sl_w],
                                                         qT[p][:, sl_w],
                                                         start=True, stop=True)
                                half_sl = slice(512 * half, 512 * half + 512)
                                for p in range(2):
                                    nc.scalar.activation(
                                        out=expT[p][:, half_sl],
                                        in_=ps_s[:, 512 * p:512 * p + 512],
                                        func=AFT.Exp, scale=0.125)
                                    nc.vector.tensor_mul(
                                        expT[p][:, half_sl],
                                        expT[p][:, half_sl],
                                        mask_sb[:, half_sl])
                            ps_av = [psV.tile([65, T], f32, tag=f"ps_av{p}",
                                              name=f"ps_av{p}") for p in range(2)]
                            for w in range(8):
                                sl = slice(128 * w, 128 * w + 128)
                                for p in range(2):
                                    h = 2 * hp + p
                                    nc.tensor.matmul(
                                        ps_av[p][:, sl],
                                        vT[w][:, 65 * h:65 * h + 65],
                                        expT[p][:, sl], start=True, stop=True)
                        else:
                            ps_av = [psV.tile([65, T], f32, tag=f"ps_av{p}",
                                              name=f"ps_av{p}") for p in range(2)]
                            for jc in range(8):
                                b = 128 * jc
                                eTj = [ep.tile([128, T], bf16, tag=f"eG{p}",
                                               name=f"eG{p}", bufs=3)
                                       for p in range(2)]
                                for ih in range(2):
                                    lo, hi = 512 * ih, 512 * ih + 512
                                    for p in range(2):
                                        ps_s = psS.tile([128, 512], f32,
                                                        tag=f"ps_s{p}",
                                                        name=f"ps_s{p}", bufs=2)
                                        nc.tensor.matmul(
                                            ps_s,
                                            kT[p][:, 128 * jc:128 * jc + 128],
                                            qT[p][:, lo:hi],
                                            start=True, stop=True)
                                        eT = eTj[p]
                                        # exp with the additive causal(+1) mask
                                        # folded in, per 512-col half:
                                        # cols < b get bias 1, cols >= b+128
                                        # get bias 0, the diagonal 128-block
                                        # gets bias 0 + {1,e} triangle mul.
                                        if b >= hi:
                                            nc.scalar.activation(
                                                out=eT[:, lo:hi], in_=ps_s,
                                                func=AFT.Exp, scale=0.125,
                                                bias=1.0)
                                        elif b + 128 <= lo:
                                            nc.scalar.activation(
                                                out=eT[:, lo:hi], in_=ps_s,
                                                func=AFT.Exp, scale=0.125)
                                        else:
                                            if b > lo:
                                                nc.scalar.activation(
                                                    out=eT[:, lo:b],
                                                    in_=ps_s[:, 0:b - lo],
                                                    func=AFT.Exp, scale=0.125,
                                                    bias=1.0)
                                            nc.scalar.activation(
                                                out=eT[:, b:hi],
                                                in_=ps_s[:, b - lo:512],
                                                func=AFT.Exp, scale=0.125)
                                            nc.vector.tensor_mul(
                                                eT[:, b:b + 128],
                                                eT[:, b:b + 128],
                                                mask_sb[:, 0:128])
                                for ih in range(2):
                                    sl = slice(512 * ih, 512 * ih + 512)
                                    for p in range(2):
                                        h = 2 * hp + p
                                        nc.tensor.matmul(
                                            ps_av[p][:, sl],
                                            vT[jc][:, 65 * h:65 * h + 65],
                                            eTj[p][:, sl],
                                            start=(jc == 0), stop=(jc == 7))
                        normalize(2 * hp, ps_av[0], tp)
                        normalize(2 * hp + 1, ps_av[1], tp)

                # out-projection into fused accumulator
                with tc.tile_pool(name=f"tmp_{br}", bufs=3) as tq, \
                     tc.tile_pool(name=f"psO_{br}", bufs=4, space="PSUM") as psO:
                    for tci in range(8):
                        for oh in range(2):
                            sl = slice(512 * oh, 512 * oh + 512)
                            ps = psO.tile([128, 512], f32, tag="psO", name="psO")
                            for fc in range(8):
                                nc.tensor.matmul(
                                    ps,
                                    attnT[fc][:, 128 * tci:128 * tci + 128],
                                    wout_sb[fc][:, sl],
                                    start=(fc == 0), stop=(fc == 7))
                            if is_local:
                                nc.vector.tensor_scalar_mul(
                                    facc[tci][:, sl], ps, scalar1=w_sc[br])
                            else:
                                tmp = tq.tile([128, 512], f32, tag="tmp", name="tmp")
                                nc.vector.tensor_scalar_mul(tmp, ps, scalar1=w_sc[br])
                                nc.vector.tensor_add(
                                    facc[tci][:, sl], facc[tci][:, sl], tmp)
                wo_es.close()

        # ---- fuse bias + LayerNorm + output ----
        with tc.tile_pool(name="ln", bufs=3) as lp:
            for tci in range(8):
                f = facc[tci]
                nc.vector.tensor_add(f, f, bcomb)
                stats = lp.tile([128, 2, 6], f32, tag="stats", name="stats")
                for sg in range(2):
                    nc.vector.bn_stats(out=stats[:, sg, :],
                                       in_=f[:, 512 * sg:512 * sg + 512])
                mv = lp.tile([128, 2], f32, tag="mv", name="mv")
                nc.vector.bn_aggr(out=mv, in_=stats)
                sq = lp.tile([128, 1], f32, tag="sq", name="sq")
                nc.scalar.activation(out=sq, in_=mv[:, 1:2], func=AFT.Sqrt,
                                     bias=eps_sb, scale=1.0)
                rstd = lp.tile([128, 1], f32, tag="rstd", name="rstd")
                nc.vector.reciprocal(out=rstd, in_=sq)
                o = lp.tile([128, D], f32, tag="o", name="o")
                nc.vector.tensor_scalar(
                    out=o, in0=f, scalar1=mv[:, 0:1], scalar2=rstd,
                    op0=mybir.AluOpType.subtract, op1=mybir.AluOpType.mult)
                nc.vector.tensor_mul(o, o, gamma_sb)
                nc.vector.tensor_add(o, o, beta_sb)
                nc.sync.dma_start(out=out_d[128 * tci:128 * tci + 128, :], in_=o)
        acc_es.close()

    nc.finalize()
    return nc


def _get_nc():
    if "nc" not in _STATE:
        _STATE["nc"] = _build_nc()
    return _STATE["nc"]


def _host_inputs(inputs):
    """Build the per-core in_maps (host-side sharding/layout/constant-folding)."""
    f = lambda a: np.asarray(a, dtype=np.float32)
    x = f(inputs["x"])
    tri = np.where(np.arange(128)[:, None] > np.arange(128)[None, :],
                   np.float32(_E), np.float32(1.0))
    mask8 = np.tile(tri, (1, 8)).astype(NBF)

    def qk_bias(b_in):
        return np.ascontiguousarray(f(b_in)[:2 * D].reshape(16, 128).T)

    def v_bias(b_in):
        return np.ascontiguousarray(f(b_in)[2 * D:].reshape(16, 64).T)

    # fold the orthogonal projection into the qkv weights (exact algebra:
    # qkv = (x @ P) @ W_in^T = x @ (P @ W_in^T))
    weffT_l = (f(inputs["proj_local"]) @ f(inputs["Wl_in"]).T).astype(NBF)
    weffT_g = (f(inputs["proj_global"]) @ f(inputs["Wg_in"]).T).astype(NBF)

    common = {
        "weffT_l": weffT_l,
        "weffT_g": weffT_g,
        "woutT_l": np.ascontiguousarray(f(inputs["Wl_out"]).T).astype(NBF),
        "woutT_g": np.ascontiguousarray(f(inputs["Wg_out"]).T).astype(NBF),
        "qkb_l": qk_bias(inputs["bl_in"]),
        "qkb_g": qk_bias(inputs["bg_in"]),
        "vb_l": v_bias(inputs["bl_in"]),
        "vb_g": v_bias(inputs["bg_in"]),
        "bout_l": f(inputs["bl_out"]).reshape(1, D),
        "bout_g": f(inputs["bg_out"]).reshape(1, D),
        "fw": f(inputs["fusion_w"]).reshape(1, 2),
        "gamma": f(inputs["ln_gamma"]).reshape(1, D),
        "beta": f(inputs["ln_beta"]).reshape(1, D),
        "mask8": mask8,
    }
    in_maps = []
    for core in range(8):
        b, t0 = core // 4, (core % 4) * T
        xT = np.ascontiguousarray(x[b, t0:t0 + T, :].T).astype(NBF)
        in_maps.append({**common, "xT": xT})
    return in_maps


def _run(inputs, trace=False):
    from concourse.bass_utils import run_bass_kernel_spmd

    nc = _get_nc()
    in_maps = _host_inputs(inputs)
    res = run_bass_kernel_spmd(nc, in_maps, core_ids=list(range(8)), trace=trace)
    x = np.asarray(inputs["x"])
    out = np.empty((2, 4096, D), np.float32)
    for core in range(8):
        b, t0 = core // 4, (core % 4) * T
        out[b, t0:t0 + T, :] = res.results[core]["out"]
    return out, res


def kernel(**inputs) -> np.ndarray:
    out, _ = _run(inputs)
    return out
